# revision 11
# baseline (speedup 1.0000x reference)
"""Causal multi-head attention on 8 trn2 NeuronCores.

Problem: B=2, S=2048, D=2048, H=16 (HD=128), fp32 in/out.
Sharding: tensor-parallel over heads — core c owns heads {2c, 2c+1} for both
batches. Each core computes its Q/K/V projections, attention for its 4
(batch, head) pairs, and a partial output projection over its head slice.
The host sums the 8 partial outputs and adds the output bias.

All matmul operands are bf16 (1 PE cycle/row, half the DMA/SBUF of fp32),
accumulation in fp32 PSUM. Q^T/K^T/V stay SBUF-resident (no DRAM spill).

Device algorithm (per core):
  Phase A (per 512-token block, ko-streamed): Q^T/K^T (head-dim on
           partitions) and V (tokens on partitions) accumulated over the 16
           contraction chunks; PSUM->SBUF copies spread over ACT/DVE/Pool.
  Phase B (per (b, qb)): S^T score tiles = K^T_chunk.T @ Q^T (keys on
           partitions), E = exp(S^T/sqrt(hd)) on ACT with causal 0/1 mask
           multiplies on diagonal tiles (DVE); the softmax denominator is
           accumulated on the Pool engine (tensor_add of E tiles) and
           reduced with ONE ones-matmul per qb, instead of a PE matmul per
           tile. ctx^T accumulates in PSUM; normalize with DVE reciprocal+
           multiply. No max-subtraction is needed: scores are O(5) so exp
           cannot overflow, and softmax is shift-invariant.
  Phase C: out-projection runs one qb behind phase B (software pipelining
           hides the exp->mask->denom->reciprocal serial chain), partial
           outputs written as bf16, alternating between both HW DMA queues
           (SP and ACT).
"""

import os

import numpy as np

import concourse.bacc as bacc
import concourse.tile as tile
from concourse import mybir
from concourse.bass_utils import run_bass_kernel_spmd


def _install_neff_cache():
    """Cache compiled NEFFs on disk keyed by BIR content hash.

    Purely a compile-time memo: identical BIR -> identical NEFF, so repeat
    runs skip the multi-minute neuronxcc compile. No effect on execution.
    """
    import hashlib
    import shutil

    import concourse.bass2jax as _b2j
    import concourse.bass_utils as _bu

    if getattr(_bu, "_neff_cache_installed", False):
        return
    cache_dir = os.environ.get("NEFF_CACHE_DIR", "/tmp/neff_cache")
    orig = _bu.compile_bir_kernel

    def cached(bir_json, tmpdir, neff_name="file.neff"):
        try:
            os.makedirs(cache_dir, exist_ok=True)
            key = hashlib.sha256(bir_json).hexdigest()[:24]
            cpath = os.path.join(cache_dir, key + ".neff")
            dst = os.path.join(tmpdir, neff_name)
            if os.path.exists(cpath):
                shutil.copy(cpath, dst)
                return dst
            out = orig(bir_json, tmpdir, neff_name)
            shutil.copy(out, cpath)
            return out
        except OSError:
            return orig(bir_json, tmpdir, neff_name)

    _bu.compile_bir_kernel = cached
    _b2j.compile_bir_kernel = cached
    _bu._neff_cache_installed = True


_install_neff_cache()

B, S, D, H = 2, 2048, 2048, 16
HD = D // H          # 128
NCORES = 8
HPC = H // NCORES    # heads per core = 2
T = B * S            # 4096 total token rows
KO = D // 128        # 16 contraction chunks
NTB = T // 512       # 8 phase-A token blocks of 512
NQB = S // 512       # 4 phase-B query blocks per batch
SCALE = 1.0 / float(np.sqrt(HD))

_built = {}


def _build(with_bias):
    f32 = mybir.dt.float32
    bf16 = mybir.dt.bfloat16
    f32r = mybir.dt.float32r

    nc = bacc.Bacc(None, target_bir_lowering=False)

    # ---- per-core DRAM parameters (host supplies per-core shards) ----
    xt_p = nc.declare_dram_parameter("XT", [KO, 128, T], bf16, False)
    wqt_p = nc.declare_dram_parameter("WQT", [KO, 128, HPC * HD], bf16, False)
    wkt_p = nc.declare_dram_parameter("WKT", [KO, 128, HPC * HD], bf16, False)
    wvt_p = nc.declare_dram_parameter("WVT", [KO, 128, HPC * HD], bf16, False)
    wot_p = nc.declare_dram_parameter("WOT", [128, HPC, D], bf16, False)
    bias_p = nc.declare_dram_parameter("BIAS", [1, 4, HPC * HD], bf16, False)
    mask_p = nc.declare_dram_parameter("MASK", [128, 4, 512], bf16, False)
    ones_p = nc.declare_dram_parameter("ONES", [128, 128], bf16, False)
    out_p = nc.declare_dram_parameter("OUT", [B, S, D], bf16, True)

    with tile.TileContext(nc) as tc:
        with tc.tile_pool(name="persist", bufs=1) as persist:
            qt_res = persist.tile([128, B, HPC, S], bf16)
            kt_res = persist.tile([128, B, HPC, S], bf16)
            # V natural layout: v_res[p, b, sc, h*HD+d] = V[b, 128*sc+p, h, d]
            v_res = persist.tile([128, B, S // 128, HPC * HD], bf16)
            wot = persist.tile([128, HPC, D], bf16)
            masks = persist.tile([128, 4, 512], bf16)
            onesr = persist.tile([128, 128], bf16)
            if with_bias:
                biasb = persist.tile([1, 4, HPC * HD], bf16)

            # ---------------- Phase A: projections ----------------
            with (
                tc.tile_pool(name="wqkv", bufs=1) as wpool,
                tc.tile_pool(name="xs", bufs=2) as xpool,
                tc.tile_pool(name="psQK", bufs=1, space="PSUM") as psQK,
                tc.tile_pool(name="psV", bufs=1, space="PSUM") as psV,
            ):
                wq = wpool.tile([128, KO, HPC * HD], bf16, tag="wq")
                wk = wpool.tile([128, KO, HPC * HD], bf16, tag="wk")
                wv = wpool.tile([128, KO, HPC * HD], bf16, tag="wv")
                # weight/constant loads ride the ACT hw queue; X rides
                # both queues. Interleave wq/wk/wv chunks so the ko-major
                # matmul stream (needs all three per ko) starts earliest.
                for g in range(4):
                    ksl = slice(g * 4, (g + 1) * 4)
                    for wt, wp in ((wq, wqt_p), (wk, wkt_p), (wv, wvt_p)):
                        nc.scalar.dma_start(
                            wt[:, ksl], wp[ksl].rearrange("ko p m -> p ko m")
                        )
                if with_bias:
                    nc.scalar.dma_start(biasb, bias_p[:])
                nc.scalar.dma_start(masks, mask_p[:])
                nc.scalar.dma_start(onesr, ones_p[:])
                nc.scalar.dma_start(wot, wot_p[:])
                warm = persist.tile([1, 8], f32)
                nc.vector.memset(warm, 1.0)
                nc.scalar.activation(
                    warm, warm, mybir.ActivationFunctionType.Exp
                )

                for tb in range(NTB):
                    b = (tb * 512) // S
                    s0 = (tb * 512) % S
                    sc0 = s0 // 128
                    xt = xpool.tile([128, KO, 512], bf16, tag="xt")
                    # tb0 streams 1-ko chunks (matches the 6-matmul/ko
                    # consumption rate at queue-ramp bandwidth); later tbs
                    # prefetch 4-ko chunks. Chunks alternate across the two
                    # HW DMA queues.
                    nch = 16 if tb == 0 else 4
                    kn = KO // nch
                    for g in range(nch):
                        ksl = slice(g * kn, (g + 1) * kn)
                        eng = nc.sync if g % 2 == 0 else nc.scalar
                        eng.dma_start(
                            xt[:, ksl],
                            xt_p[ksl, :, tb * 512 : (tb + 1) * 512].rearrange(
                                "ko p t -> p ko t"
                            ),
                        )

                    psq = [
                        psQK.tile([128, 512], f32, tag=f"q{h}", name=f"psq{h}") for h in range(HPC)
                    ]
                    psk = [
                        psQK.tile([128, 512], f32, tag=f"k{h}", name=f"psk{h}") for h in range(HPC)
                    ]
                    # one PSUM bank per V accumulation chain: start=True
                    # clears has_written for the WHOLE bank, so chains must
                    # not share banks
                    psvs = [
                        psV.tile([128, 512], f32, tag=f"v{i}", name=f"psv{i}")
                        for i in range(4)
                    ]
                    # ko-major so each arriving X chunk feeds ~1.3us of PE
                    # work immediately (keeps the PE dense from t~2us).
                    for ko in range(KO):
                        st = ko == 0
                        sp = (ko == KO - 1) and not with_bias
                        for h in range(HPC):
                            nc.tensor.matmul(
                                psq[h],
                                lhsT=wq[:, ko, h * HD : (h + 1) * HD],
                                rhs=xt[:, ko],
                                start=st,
                                stop=sp,
                            )
                        for h in range(HPC):
                            nc.tensor.matmul(
                                psk[h],
                                lhsT=wk[:, ko, h * HD : (h + 1) * HD],
                                rhs=xt[:, ko],
                                start=st,
                                stop=sp,
                            )
                        for tsub in range(4):
                            nc.tensor.matmul(
                                psvs[tsub][:, :256],
                                lhsT=xt[:, ko, tsub * 128 : (tsub + 1) * 128],
                                rhs=wv[:, ko],
                                start=st,
                                stop=sp,
                            )
                    if with_bias:
                        ones512 = masks[0:1, 0, :]  # [1,512] of exact ones
                        for h in range(HPC):
                            nc.tensor.matmul(
                                psq[h],
                                lhsT=biasb[:, 0, h * HD : (h + 1) * HD],
                                rhs=ones512,
                                start=False,
                                stop=True,
                            )
                            nc.tensor.matmul(
                                psk[h],
                                lhsT=biasb[:, 1, h * HD : (h + 1) * HD],
                                rhs=ones512,
                                start=False,
                                stop=True,
                            )
                        for tsub in range(4):
                            nc.tensor.matmul(
                                psvs[tsub][:, :256],
                                lhsT=ones512[:, :128],
                                rhs=biasb[:, 2],
                                start=False,
                                stop=True,
                            )
                    # PSUM->SBUF copies spread across ACT + DVE (Pool
                    # cannot access PSUM)
                    for h in range(HPC):
                        nc.scalar.copy(qt_res[:, b, h, s0 : s0 + 512], psq[h])
                        nc.vector.tensor_copy(kt_res[:, b, h, s0 : s0 + 512], psk[h])
                    for tsub in range(4):
                        nc.vector.tensor_copy(
                            v_res[:, b, sc0 + tsub, :], psvs[tsub][:, :256]
                        )

            # ------------- Phase B + C: attention + out projection -------------
            with (
                tc.tile_pool(name="epool", bufs=34) as epool,
                tc.tile_pool(name="ctxp", bufs=2) as ctxp,
                tc.tile_pool(name="recp", bufs=3) as recp,
                tc.tile_pool(name="obp", bufs=4) as obp,
                tc.tile_pool(name="psS", bufs=2, space="PSUM") as psS,
                tc.tile_pool(name="psC", bufs=2, space="PSUM") as psC,
                tc.tile_pool(name="psD", bufs=2, space="PSUM") as psD,
                tc.tile_pool(name="psO", bufs=2, space="PSUM") as psO,
            ):
                dma_eng = [nc.sync, nc.scalar]

                def emit_outproj(b, qb, ctxs, cur_nk):
                    for qc in range(4 * qb, 4 * (qb + 1)):
                        for oc in range(D // 512):
                            pso = psO.tile([128, 512], f32, tag="o")
                            for h in range(HPC):
                                nc.tensor.matmul(
                                    pso,
                                    lhsT=ctxs[h][:, qc * 128 : (qc + 1) * 128],
                                    rhs=wot[:, h, oc * 512 : (oc + 1) * 512],
                                    start=(h == 0),
                                    stop=(h == HPC - 1),
                                )
                            ob = obp.tile([128, 512], bf16, tag="ob")
                            # split the PSUM->SBUF casts between DVE and ACT
                            # by the surrounding window's load: in big
                            # windows ACT is exp-bound, in small ones DVE is
                            # the tighter engine
                            k = (qc * 4 + oc) % 4
                            use_act = (k % 2 == 1) if cur_nk <= 8 else (k == 3)
                            if use_act:
                                nc.scalar.copy(ob, pso)
                            else:
                                nc.vector.tensor_copy(ob, pso)
                            dma_eng[(qc + oc) % 2].dma_start(
                                out_p[
                                    b,
                                    qc * 128 : (qc + 1) * 128,
                                    oc * 512 : (oc + 1) * 512,
                                ],
                                ob,
                            )

                pending = None
                for b in range(B):
                    ctxs = [
                        ctxp.tile([128, S], bf16, tag=f"ctx{h}", name=f"ctx{h}") for h in range(HPC)
                    ]
                    for qb in range(NQB):
                        nk = 4 * (qb + 1)

                        def cut(t):
                            # diagonal tiles: queries < 128*i are fully
                            # masked for key chunk i -> skip those columns
                            return 128 * (t - 4 * qb) if t >= 4 * qb else 0

                        ess = [[], []]
                        pscs = [
                            psC.tile([128, 512], f32, tag="c", name=f"psc{h}") for h in range(HPC)
                        ]
                        psds = [
                            psD.tile([128, 512], f32, tag="d", name=f"psd{h}") for h in range(HPC)
                        ]
                        # Three interleaved chains at lags 0/1/2: score
                        # matmuls, denominator (ones) matmuls, ctx matmuls.
                        # 6 matmuls per step keep the PE busy past the exp
                        # latency, and the denominator rides the PE without
                        # a separate reduction pass.
                        for t in range(nk + 2):
                            if t < nk:
                                c0 = cut(t)
                                for h in range(HPC):
                                    pss = psS.tile([128, 512], f32, tag="s")
                                    nc.tensor.matmul(
                                        pss[:, c0:],
                                        lhsT=kt_res[:, b, h, t * 128 : (t + 1) * 128],
                                        rhs=qt_res[
                                            :, b, h,
                                            qb * 512 + c0 : (qb + 1) * 512,
                                        ],
                                        start=True,
                                        stop=True,
                                    )
                                    e = epool.tile([128, 512], bf16, tag="e")
                                    nc.scalar.activation(
                                        e[:, c0:], pss[:, c0:],
                                        mybir.ActivationFunctionType.Exp,
                                        scale=SCALE,
                                    )
                                    if t >= 4 * qb:
                                        # full-width: also zeroes the stale
                                        # (skipped) columns of the ring tile
                                        nc.vector.tensor_mul(
                                            e, e, masks[:, t - 4 * qb]
                                        )
                                    ess[h].append(e)
                            td = t - 1
                            if 0 <= td < nk:
                                c0 = cut(td)
                                for h in range(HPC):
                                    nc.tensor.matmul(
                                        psds[h][:, c0:],
                                        lhsT=onesr,
                                        rhs=ess[h][td][:, c0:],
                                        start=(td == 0),
                                        stop=(td == nk - 1),
                                    )
                            tc_ = t - 2
                            if 0 <= tc_ < nk:
                                c0 = cut(tc_)
                                for h in range(HPC):
                                    nc.tensor.matmul(
                                        pscs[h][:, c0:],
                                        lhsT=v_res[:, b, tc_, h * HD : (h + 1) * HD],
                                        rhs=ess[h][tc_][:, c0:],
                                        start=(tc_ == 0),
                                        stop=(tc_ == nk - 1),
                                    )
                        # --- normalize ---
                        for h in range(HPC):
                            rec = recp.tile([128, 512], f32, tag="rec")
                            nc.vector.reciprocal_approx_fast(rec, psds[h])
                            nc.vector.tensor_mul(
                                ctxs[h][:, qb * 512 : (qb + 1) * 512], pscs[h], rec
                            )
                        # --- out-projection for the previous qb (pipelined:
                        # its ctx was normalized during this qb's matmuls) ---
                        if pending is not None:
                            emit_outproj(*pending, nk)
                        pending = (b, qb, ctxs)
                emit_outproj(*pending, 16)

    nc.finalize()
    return nc


def _get_nc(with_bias=False):
    if with_bias not in _built:
        _built[with_bias] = _build(with_bias)
    return _built[with_bias]


def kernel(hidden_states, attention_mask, Wq, bq, Wk, bk, Wv, bv, Wo, bo):
    import ml_dtypes

    bf16 = ml_dtypes.bfloat16

    hidden_states = np.asarray(hidden_states, dtype=np.float32)
    Wq, Wk, Wv, Wo = (np.asarray(w, dtype=np.float32) for w in (Wq, Wk, Wv, Wo))
    bq, bk, bv, bo = (np.asarray(v, dtype=np.float32) for v in (bq, bk, bv, bo))

    with_bias = bool(np.any(bq) or np.any(bk) or np.any(bv))

    x = hidden_states.reshape(T, D)
    # [KO, 128, T]: XT[ko, p, t] = x[t, 128*ko + p]
    xt = np.ascontiguousarray(x.T).reshape(KO, 128, T).astype(bf16)

    # causal 0/1 masks for the 4 diagonal-tile offsets: mask[p, i, f] = p + 128*i <= f
    p_idx = np.arange(128)[:, None, None]
    i_idx = np.arange(4)[None, :, None]
    f_idx = np.arange(512)[None, None, :]
    mask = (p_idx + 128 * i_idx <= f_idx).astype(bf16)
    ones = np.ones((128, 128), dtype=bf16)

    in_maps = []
    for c in range(NCORES):
        rows = slice(c * HPC * HD, (c + 1) * HPC * HD)
        wqt = np.ascontiguousarray(Wq[rows, :].T).reshape(KO, 128, HPC * HD)
        wkt = np.ascontiguousarray(Wk[rows, :].T).reshape(KO, 128, HPC * HD)
        wvt = np.ascontiguousarray(Wv[rows, :].T).reshape(KO, 128, HPC * HD)
        # WOT[p, h, n] = Wo[n, c*256 + h*128 + p]
        wot = np.ascontiguousarray(
            Wo[:, rows].T.reshape(HPC, 128, D).transpose(1, 0, 2)
        )
        bias = np.stack([bq[rows], bk[rows], bv[rows], np.zeros(HPC * HD, np.float32)])[
            None
        ]
        in_maps.append(
            {
                "XT": xt,
                "WQT": wqt.astype(bf16),
                "WKT": wkt.astype(bf16),
                "WVT": wvt.astype(bf16),
                "WOT": wot.astype(bf16),
                "BIAS": bias.astype(bf16),
                "MASK": mask,
                "ONES": ones,
            }
        )

    res = run_bass_kernel_spmd(_get_nc(with_bias), in_maps, list(range(NCORES)))
    out = res.results[0]["OUT"].astype(np.float32)
    for c in range(1, NCORES):
        out += res.results[c]["OUT"].astype(np.float32)
    out += bo
    return out


# revision 12
# speedup vs baseline: 1.0499x; 1.0499x over previous
"""Causal multi-head attention on 8 trn2 NeuronCores.

Problem: B=2, S=2048, D=2048, H=16 (HD=128), fp32 in/out.
Sharding: tensor-parallel over heads — core c owns heads {2c, 2c+1} for both
batches. Each core computes its Q/K/V projections, attention for its 4
(batch, head) pairs, and a partial output projection over its head slice.
The host sums the 8 partial outputs and adds the output bias.

All matmul operands are bf16 (1 PE cycle/row, half the DMA/SBUF of fp32),
accumulation in fp32 PSUM. Q^T/K^T/V stay SBUF-resident (no DRAM spill).

Device algorithm (per core):
  Phase A (per 512-token block, ko-streamed): Q^T/K^T (head-dim on
           partitions) and V (tokens on partitions) accumulated over the 16
           contraction chunks; PSUM->SBUF copies spread over ACT/DVE/Pool.
  Phase B (per (b, qb)): S^T score tiles = K^T_chunk.T @ Q^T (keys on
           partitions), E = exp(S^T/sqrt(hd)) on ACT with causal 0/1 mask
           multiplies on diagonal tiles (DVE); the softmax denominator is
           accumulated on the Pool engine (tensor_add of E tiles) and
           reduced with ONE ones-matmul per qb, instead of a PE matmul per
           tile. ctx^T accumulates in PSUM; normalize with DVE reciprocal+
           multiply. No max-subtraction is needed: scores are O(5) so exp
           cannot overflow, and softmax is shift-invariant.
  Phase C: out-projection runs one qb behind phase B (software pipelining
           hides the exp->mask->denom->reciprocal serial chain), partial
           outputs written as bf16, alternating between both HW DMA queues
           (SP and ACT).
"""

import os

import numpy as np

import concourse.bacc as bacc
import concourse.tile as tile
from concourse import mybir
from concourse.bass_utils import run_bass_kernel_spmd


def _install_neff_cache():
    """Cache compiled NEFFs on disk keyed by BIR content hash.

    Purely a compile-time memo: identical BIR -> identical NEFF, so repeat
    runs skip the multi-minute neuronxcc compile. No effect on execution.
    """
    import hashlib
    import shutil

    import concourse.bass2jax as _b2j
    import concourse.bass_utils as _bu

    if getattr(_bu, "_neff_cache_installed", False):
        return
    cache_dir = os.environ.get("NEFF_CACHE_DIR", "/tmp/neff_cache")
    orig = _bu.compile_bir_kernel

    def cached(bir_json, tmpdir, neff_name="file.neff"):
        try:
            os.makedirs(cache_dir, exist_ok=True)
            key = hashlib.sha256(bir_json).hexdigest()[:24]
            cpath = os.path.join(cache_dir, key + ".neff")
            dst = os.path.join(tmpdir, neff_name)
            if os.path.exists(cpath):
                shutil.copy(cpath, dst)
                return dst
            out = orig(bir_json, tmpdir, neff_name)
            shutil.copy(out, cpath)
            return out
        except OSError:
            return orig(bir_json, tmpdir, neff_name)

    _bu.compile_bir_kernel = cached
    _b2j.compile_bir_kernel = cached
    _bu._neff_cache_installed = True


_install_neff_cache()

B, S, D, H = 2, 2048, 2048, 16
HD = D // H          # 128
NCORES = 8
HPC = H // NCORES    # heads per core = 2
T = B * S            # 4096 total token rows
KO = D // 128        # 16 contraction chunks
NTB = T // 512       # 8 phase-A token blocks of 512
NQB = S // 512       # 4 phase-B query blocks per batch
SCALE = 1.0 / float(np.sqrt(HD))

_built = {}


def _build(with_bias):
    f32 = mybir.dt.float32
    bf16 = mybir.dt.bfloat16
    f32r = mybir.dt.float32r

    nc = bacc.Bacc(None, target_bir_lowering=False)

    # ---- per-core DRAM parameters (host supplies per-core shards) ----
    xt_p = nc.declare_dram_parameter("XT", [KO, 128, T], bf16, False)
    wqt_p = nc.declare_dram_parameter("WQT", [KO, 128, HPC * HD], bf16, False)
    wkt_p = nc.declare_dram_parameter("WKT", [KO, 128, HPC * HD], bf16, False)
    wvt_p = nc.declare_dram_parameter("WVT", [KO, 128, HPC * HD], bf16, False)
    wot_p = nc.declare_dram_parameter("WOT", [128, HPC, D], bf16, False)
    bias_p = nc.declare_dram_parameter("BIAS", [1, 4, HPC * HD], bf16, False)
    mask_p = nc.declare_dram_parameter("MASK", [128, 4, 512], bf16, False)
    ones_p = nc.declare_dram_parameter("ONES", [128, 128], bf16, False)
    out_p = nc.declare_dram_parameter("OUT", [B, S, D], bf16, True)

    with tile.TileContext(nc) as tc:
        with tc.tile_pool(name="persist", bufs=1) as persist:
            qt_res = persist.tile([128, B, HPC, S], bf16)
            kt_res = persist.tile([128, B, HPC, S], bf16)
            # V natural layout: v_res[p, b, sc, h*HD+d] = V[b, 128*sc+p, h, d]
            v_res = persist.tile([128, B, S // 128, HPC * HD], bf16)
            wot = persist.tile([128, HPC, D], bf16)
            masks = persist.tile([128, 4, 512], bf16)
            onesr = persist.tile([128, 128], bf16)
            if with_bias:
                biasb = persist.tile([1, 4, HPC * HD], bf16)

            # ---------------- Phase A: projections ----------------
            with (
                tc.tile_pool(name="wqkv", bufs=1) as wpool,
                tc.tile_pool(name="xs", bufs=2) as xpool,
                tc.tile_pool(name="psQK", bufs=1, space="PSUM") as psQK,
                tc.tile_pool(name="psV", bufs=1, space="PSUM") as psV,
            ):
                wq = wpool.tile([128, KO, HPC * HD], bf16, tag="wq")
                wk = wpool.tile([128, KO, HPC * HD], bf16, tag="wk")
                wv = wpool.tile([128, KO, HPC * HD], bf16, tag="wv")
                # weight/constant loads ride the ACT hw queue; X rides
                # both queues. Interleave wq/wk/wv chunks so the ko-major
                # matmul stream (needs all three per ko) starts earliest.
                for g in range(4):
                    ksl = slice(g * 4, (g + 1) * 4)
                    for wt, wp in ((wq, wqt_p), (wk, wkt_p), (wv, wvt_p)):
                        nc.scalar.dma_start(
                            wt[:, ksl], wp[ksl].rearrange("ko p m -> p ko m")
                        )
                if with_bias:
                    nc.scalar.dma_start(biasb, bias_p[:])
                nc.scalar.dma_start(masks, mask_p[:])
                nc.scalar.dma_start(onesr, ones_p[:])
                nc.scalar.dma_start(wot, wot_p[:])
                warm = persist.tile([1, 8], f32)
                nc.vector.memset(warm, 1.0)
                nc.scalar.activation(
                    warm, warm, mybir.ActivationFunctionType.Exp
                )

                for tb in range(NTB):
                    b = (tb * 512) // S
                    s0 = (tb * 512) % S
                    sc0 = s0 // 128
                    xt = xpool.tile([128, KO, 512], bf16, tag="xt")
                    # tb0 streams 1-ko chunks (matches the 6-matmul/ko
                    # consumption rate at queue-ramp bandwidth); later tbs
                    # prefetch 4-ko chunks. Chunks alternate across the two
                    # HW DMA queues.
                    nch = 16 if tb == 0 else 4
                    kn = KO // nch
                    for g in range(nch):
                        ksl = slice(g * kn, (g + 1) * kn)
                        # the scalar queue carries ~3MB of weights first, so
                        # X rides sync-only until they drain (tb0/tb1)
                        eng = nc.sync if (tb < 2 or g % 2 == 0) else nc.scalar
                        eng.dma_start(
                            xt[:, ksl],
                            xt_p[ksl, :, tb * 512 : (tb + 1) * 512].rearrange(
                                "ko p t -> p ko t"
                            ),
                        )

                    psq = [
                        psQK.tile([128, 512], f32, tag=f"q{h}", name=f"psq{h}") for h in range(HPC)
                    ]
                    psk = [
                        psQK.tile([128, 512], f32, tag=f"k{h}", name=f"psk{h}") for h in range(HPC)
                    ]
                    # one PSUM bank per V accumulation chain: start=True
                    # clears has_written for the WHOLE bank, so chains must
                    # not share banks
                    psvs = [
                        psV.tile([128, 512], f32, tag=f"v{i}", name=f"psv{i}")
                        for i in range(4)
                    ]
                    # ko-major so each arriving X chunk feeds ~1.3us of PE
                    # work immediately (keeps the PE dense from t~2us).
                    for ko in range(KO):
                        st = ko == 0
                        sp = (ko == KO - 1) and not with_bias
                        for h in range(HPC):
                            nc.tensor.matmul(
                                psq[h],
                                lhsT=wq[:, ko, h * HD : (h + 1) * HD],
                                rhs=xt[:, ko],
                                start=st,
                                stop=sp,
                            )
                        for h in range(HPC):
                            nc.tensor.matmul(
                                psk[h],
                                lhsT=wk[:, ko, h * HD : (h + 1) * HD],
                                rhs=xt[:, ko],
                                start=st,
                                stop=sp,
                            )
                        for tsub in range(4):
                            nc.tensor.matmul(
                                psvs[tsub][:, :256],
                                lhsT=xt[:, ko, tsub * 128 : (tsub + 1) * 128],
                                rhs=wv[:, ko],
                                start=st,
                                stop=sp,
                            )
                    if with_bias:
                        ones512 = masks[0:1, 0, :]  # [1,512] of exact ones
                        for h in range(HPC):
                            nc.tensor.matmul(
                                psq[h],
                                lhsT=biasb[:, 0, h * HD : (h + 1) * HD],
                                rhs=ones512,
                                start=False,
                                stop=True,
                            )
                            nc.tensor.matmul(
                                psk[h],
                                lhsT=biasb[:, 1, h * HD : (h + 1) * HD],
                                rhs=ones512,
                                start=False,
                                stop=True,
                            )
                        for tsub in range(4):
                            nc.tensor.matmul(
                                psvs[tsub][:, :256],
                                lhsT=ones512[:, :128],
                                rhs=biasb[:, 2],
                                start=False,
                                stop=True,
                            )
                    # PSUM->SBUF copies spread across ACT + DVE (Pool
                    # cannot access PSUM)
                    for h in range(HPC):
                        nc.scalar.copy(qt_res[:, b, h, s0 : s0 + 512], psq[h])
                        nc.vector.tensor_copy(kt_res[:, b, h, s0 : s0 + 512], psk[h])
                    for tsub in range(4):
                        nc.vector.tensor_copy(
                            v_res[:, b, sc0 + tsub, :], psvs[tsub][:, :256]
                        )

            # ------------- Phase B + C: attention + out projection -------------
            with (
                tc.tile_pool(name="epool", bufs=34) as epool,
                tc.tile_pool(name="ctxp", bufs=2) as ctxp,
                tc.tile_pool(name="recp", bufs=3) as recp,
                tc.tile_pool(name="obp", bufs=4) as obp,
                tc.tile_pool(name="psS", bufs=2, space="PSUM") as psS,
                tc.tile_pool(name="psC", bufs=2, space="PSUM") as psC,
                tc.tile_pool(name="psD", bufs=2, space="PSUM") as psD,
                tc.tile_pool(name="psO", bufs=2, space="PSUM") as psO,
            ):
                dma_eng = [nc.sync, nc.scalar]

                def emit_outproj(b, qb, ctxs, cur_nk):
                    for qc in range(4 * qb, 4 * (qb + 1)):
                        for oc in range(D // 512):
                            pso = psO.tile([128, 512], f32, tag="o")
                            for h in range(HPC):
                                nc.tensor.matmul(
                                    pso,
                                    lhsT=ctxs[h][:, qc * 128 : (qc + 1) * 128],
                                    rhs=wot[:, h, oc * 512 : (oc + 1) * 512],
                                    start=(h == 0),
                                    stop=(h == HPC - 1),
                                )
                            ob = obp.tile([128, 512], bf16, tag="ob")
                            # split the PSUM->SBUF casts between DVE and ACT
                            # by the surrounding window's load: in big
                            # windows ACT is exp-bound, in small ones DVE is
                            # the tighter engine
                            k = (qc * 4 + oc) % 4
                            use_act = (k % 2 == 1) if cur_nk <= 8 else (k == 3)
                            if use_act:
                                nc.scalar.copy(ob, pso)
                            else:
                                nc.vector.tensor_copy(ob, pso)
                            dma_eng[(qc + oc) % 2].dma_start(
                                out_p[
                                    b,
                                    qc * 128 : (qc + 1) * 128,
                                    oc * 512 : (oc + 1) * 512,
                                ],
                                ob,
                            )

                pending = None
                for b in range(B):
                    ctxs = [
                        ctxp.tile([128, S], bf16, tag=f"ctx{h}", name=f"ctx{h}") for h in range(HPC)
                    ]
                    for qb in range(NQB):
                        nk = 4 * (qb + 1)

                        def cut(t):
                            # diagonal tiles: queries < 128*i are fully
                            # masked for key chunk i -> skip those columns
                            return 128 * (t - 4 * qb) if t >= 4 * qb else 0

                        ess = [[], []]
                        pscs = [
                            psC.tile([128, 512], f32, tag="c", name=f"psc{h}") for h in range(HPC)
                        ]
                        psds = [
                            psD.tile([128, 512], f32, tag="d", name=f"psd{h}") for h in range(HPC)
                        ]
                        # Three interleaved chains at lags 0/1/2: score
                        # matmuls, denominator (ones) matmuls, ctx matmuls.
                        # 6 matmuls per step keep the PE busy past the exp
                        # latency, and the denominator rides the PE without
                        # a separate reduction pass.
                        for t in range(nk + 2):
                            if t < nk:
                                c0 = cut(t)
                                for h in range(HPC):
                                    pss = psS.tile([128, 512], f32, tag="s")
                                    nc.tensor.matmul(
                                        pss[:, c0:],
                                        lhsT=kt_res[:, b, h, t * 128 : (t + 1) * 128],
                                        rhs=qt_res[
                                            :, b, h,
                                            qb * 512 + c0 : (qb + 1) * 512,
                                        ],
                                        start=True,
                                        stop=True,
                                    )
                                    e = epool.tile([128, 512], bf16, tag="e")
                                    nc.scalar.activation(
                                        e[:, c0:], pss[:, c0:],
                                        mybir.ActivationFunctionType.Exp,
                                        scale=SCALE,
                                    )
                                    if t >= 4 * qb:
                                        # full-width: also zeroes the stale
                                        # (skipped) columns of the ring tile
                                        nc.vector.tensor_mul(
                                            e, e, masks[:, t - 4 * qb]
                                        )
                                    ess[h].append(e)
                            td = t - 1
                            if 0 <= td < nk:
                                c0 = cut(td)
                                for h in range(HPC):
                                    nc.tensor.matmul(
                                        psds[h][:, c0:],
                                        lhsT=onesr,
                                        rhs=ess[h][td][:, c0:],
                                        start=(td == 0),
                                        stop=(td == nk - 1),
                                    )
                            tc_ = t - 2
                            if 0 <= tc_ < nk:
                                c0 = cut(tc_)
                                for h in range(HPC):
                                    nc.tensor.matmul(
                                        pscs[h][:, c0:],
                                        lhsT=v_res[:, b, tc_, h * HD : (h + 1) * HD],
                                        rhs=ess[h][tc_][:, c0:],
                                        start=(tc_ == 0),
                                        stop=(tc_ == nk - 1),
                                    )
                        # --- normalize ---
                        for h in range(HPC):
                            rec = recp.tile([128, 512], f32, tag="rec")
                            nc.vector.reciprocal_approx_fast(rec, psds[h])
                            nc.vector.tensor_mul(
                                ctxs[h][:, qb * 512 : (qb + 1) * 512], pscs[h], rec
                            )
                        # --- out-projection for the previous qb (pipelined:
                        # its ctx was normalized during this qb's matmuls) ---
                        if pending is not None:
                            emit_outproj(*pending, nk)
                        pending = (b, qb, ctxs)
                emit_outproj(*pending, 16)

    nc.finalize()
    return nc


def _get_nc(with_bias=False):
    if with_bias not in _built:
        _built[with_bias] = _build(with_bias)
    return _built[with_bias]


def kernel(hidden_states, attention_mask, Wq, bq, Wk, bk, Wv, bv, Wo, bo):
    import ml_dtypes

    bf16 = ml_dtypes.bfloat16

    hidden_states = np.asarray(hidden_states, dtype=np.float32)
    Wq, Wk, Wv, Wo = (np.asarray(w, dtype=np.float32) for w in (Wq, Wk, Wv, Wo))
    bq, bk, bv, bo = (np.asarray(v, dtype=np.float32) for v in (bq, bk, bv, bo))

    with_bias = bool(np.any(bq) or np.any(bk) or np.any(bv))

    x = hidden_states.reshape(T, D)
    # [KO, 128, T]: XT[ko, p, t] = x[t, 128*ko + p]
    xt = np.ascontiguousarray(x.T).reshape(KO, 128, T).astype(bf16)

    # causal 0/1 masks for the 4 diagonal-tile offsets: mask[p, i, f] = p + 128*i <= f
    p_idx = np.arange(128)[:, None, None]
    i_idx = np.arange(4)[None, :, None]
    f_idx = np.arange(512)[None, None, :]
    mask = (p_idx + 128 * i_idx <= f_idx).astype(bf16)
    ones = np.ones((128, 128), dtype=bf16)

    in_maps = []
    for c in range(NCORES):
        rows = slice(c * HPC * HD, (c + 1) * HPC * HD)
        wqt = np.ascontiguousarray(Wq[rows, :].T).reshape(KO, 128, HPC * HD)
        wkt = np.ascontiguousarray(Wk[rows, :].T).reshape(KO, 128, HPC * HD)
        wvt = np.ascontiguousarray(Wv[rows, :].T).reshape(KO, 128, HPC * HD)
        # WOT[p, h, n] = Wo[n, c*256 + h*128 + p]
        wot = np.ascontiguousarray(
            Wo[:, rows].T.reshape(HPC, 128, D).transpose(1, 0, 2)
        )
        bias = np.stack([bq[rows], bk[rows], bv[rows], np.zeros(HPC * HD, np.float32)])[
            None
        ]
        in_maps.append(
            {
                "XT": xt,
                "WQT": wqt.astype(bf16),
                "WKT": wkt.astype(bf16),
                "WVT": wvt.astype(bf16),
                "WOT": wot.astype(bf16),
                "BIAS": bias.astype(bf16),
                "MASK": mask,
                "ONES": ones,
            }
        )

    res = run_bass_kernel_spmd(_get_nc(with_bias), in_maps, list(range(NCORES)))
    out = res.results[0]["OUT"].astype(np.float32)
    for c in range(1, NCORES):
        out += res.results[c]["OUT"].astype(np.float32)
    out += bo
    return out


# revision 13
# speedup vs baseline: 1.0686x; 1.0178x over previous
"""Causal multi-head attention on 8 trn2 NeuronCores.

Problem: B=2, S=2048, D=2048, H=16 (HD=128), fp32 in/out.
Sharding: tensor-parallel over heads — core c owns heads {2c, 2c+1} for both
batches. Each core computes its Q/K/V projections, attention for its 4
(batch, head) pairs, and a partial output projection over its head slice.
The host sums the 8 partial outputs and adds the output bias.

All matmul operands are bf16 (1 PE cycle/row, half the DMA/SBUF of fp32),
accumulation in fp32 PSUM. Q^T/K^T/V stay SBUF-resident (no DRAM spill).

Device algorithm (per core):
  Phase A (per 512-token block, ko-streamed): Q^T/K^T (head-dim on
           partitions) and V (tokens on partitions) accumulated over the 16
           contraction chunks; PSUM->SBUF copies spread over ACT/DVE/Pool.
  Phase B (per (b, qb)): S^T score tiles = K^T_chunk.T @ Q^T (keys on
           partitions), E = exp(S^T/sqrt(hd)) on ACT with causal 0/1 mask
           multiplies on diagonal tiles (DVE); the softmax denominator is
           accumulated on the Pool engine (tensor_add of E tiles) and
           reduced with ONE ones-matmul per qb, instead of a PE matmul per
           tile. ctx^T accumulates in PSUM; normalize with DVE reciprocal+
           multiply. No max-subtraction is needed: scores are O(5) so exp
           cannot overflow, and softmax is shift-invariant.
  Phase C: out-projection runs one qb behind phase B (software pipelining
           hides the exp->mask->denom->reciprocal serial chain), partial
           outputs written as bf16, alternating between both HW DMA queues
           (SP and ACT).
"""

import os

import numpy as np

import concourse.bacc as bacc
import concourse.tile as tile
from concourse import mybir
from concourse.bass_utils import run_bass_kernel_spmd


def _install_neff_cache():
    """Cache compiled NEFFs on disk keyed by BIR content hash.

    Purely a compile-time memo: identical BIR -> identical NEFF, so repeat
    runs skip the multi-minute neuronxcc compile. No effect on execution.
    """
    import hashlib
    import shutil

    import concourse.bass2jax as _b2j
    import concourse.bass_utils as _bu

    if getattr(_bu, "_neff_cache_installed", False):
        return
    cache_dir = os.environ.get("NEFF_CACHE_DIR", "/tmp/neff_cache")
    orig = _bu.compile_bir_kernel

    def cached(bir_json, tmpdir, neff_name="file.neff"):
        try:
            os.makedirs(cache_dir, exist_ok=True)
            key = hashlib.sha256(bir_json).hexdigest()[:24]
            cpath = os.path.join(cache_dir, key + ".neff")
            dst = os.path.join(tmpdir, neff_name)
            if os.path.exists(cpath):
                shutil.copy(cpath, dst)
                return dst
            out = orig(bir_json, tmpdir, neff_name)
            shutil.copy(out, cpath)
            return out
        except OSError:
            return orig(bir_json, tmpdir, neff_name)

    _bu.compile_bir_kernel = cached
    _b2j.compile_bir_kernel = cached
    _bu._neff_cache_installed = True


_install_neff_cache()

B, S, D, H = 2, 2048, 2048, 16
HD = D // H          # 128
NCORES = 8
HPC = H // NCORES    # heads per core = 2
T = B * S            # 4096 total token rows
KO = D // 128        # 16 contraction chunks
NTB = T // 512       # 8 phase-A token blocks of 512
NQB = S // 512       # 4 phase-B query blocks per batch
SCALE = 1.0 / float(np.sqrt(HD))

_built = {}


def _build(with_bias):
    f32 = mybir.dt.float32
    bf16 = mybir.dt.bfloat16
    f32r = mybir.dt.float32r

    nc = bacc.Bacc(None, target_bir_lowering=False)

    # ---- per-core DRAM parameters (host supplies per-core shards) ----
    xt_p = nc.declare_dram_parameter("XT", [KO, 128, T], bf16, False)
    wqt_p = nc.declare_dram_parameter("WQT", [KO, 128, HPC * HD], bf16, False)
    wkt_p = nc.declare_dram_parameter("WKT", [KO, 128, HPC * HD], bf16, False)
    wvt_p = nc.declare_dram_parameter("WVT", [KO, 128, HPC * HD], bf16, False)
    wot_p = nc.declare_dram_parameter("WOT", [128, HPC, D], bf16, False)
    bias_p = nc.declare_dram_parameter("BIAS", [1, 4, HPC * HD], bf16, False)
    mask_p = nc.declare_dram_parameter("MASK", [128, 4, 512], bf16, False)
    ones_p = nc.declare_dram_parameter("ONES", [128, 128], bf16, False)
    out_p = nc.declare_dram_parameter("OUT", [B, S, D], bf16, True)

    with tile.TileContext(nc) as tc:
        with tc.tile_pool(name="persist", bufs=1) as persist:
            qt_res = persist.tile([128, B, HPC, S], bf16)
            kt_res = persist.tile([128, B, HPC, S], bf16)
            # V natural layout: v_res[p, b, sc, h*HD+d] = V[b, 128*sc+p, h, d]
            v_res = persist.tile([128, B, S // 128, HPC * HD], bf16)
            wot = persist.tile([128, HPC, D], bf16)
            masks = persist.tile([128, 4, 512], bf16)
            onesr = persist.tile([128, 128], bf16)
            if with_bias:
                biasb = persist.tile([1, 4, HPC * HD], bf16)

            # ---------------- Phase A: projections ----------------
            with (
                tc.tile_pool(name="wqkv", bufs=1) as wpool,
                tc.tile_pool(name="xs", bufs=2) as xpool,
                tc.tile_pool(name="psQK", bufs=1, space="PSUM") as psQK,
                tc.tile_pool(name="psV", bufs=1, space="PSUM") as psV,
            ):
                wq = wpool.tile([128, KO, HPC * HD], bf16, tag="wq")
                wk = wpool.tile([128, KO, HPC * HD], bf16, tag="wk")
                wv = wpool.tile([128, KO, HPC * HD], bf16, tag="wv")
                # weight/constant loads ride the ACT hw queue; X rides
                # both queues. Interleave wq/wk/wv chunks so the ko-major
                # matmul stream (needs all three per ko) starts earliest.
                for g in range(4):
                    ksl = slice(g * 4, (g + 1) * 4)
                    for wt, wp in ((wq, wqt_p), (wk, wkt_p), (wv, wvt_p)):
                        nc.scalar.dma_start(
                            wt[:, ksl], wp[ksl].rearrange("ko p m -> p ko m")
                        )
                if with_bias:
                    nc.scalar.dma_start(biasb, bias_p[:])
                nc.scalar.dma_start(masks, mask_p[:])
                nc.scalar.dma_start(onesr, ones_p[:])
                nc.scalar.dma_start(wot, wot_p[:])
                warm = persist.tile([1, 8], f32)
                nc.vector.memset(warm, 1.0)
                nc.scalar.activation(
                    warm, warm, mybir.ActivationFunctionType.Exp
                )

                for tb in range(NTB):
                    b = (tb * 512) // S
                    s0 = (tb * 512) % S
                    sc0 = s0 // 128
                    xt = xpool.tile([128, KO, 512], bf16, tag="xt")
                    # tb0 streams 1-ko chunks (matches the 6-matmul/ko
                    # consumption rate at queue-ramp bandwidth); later tbs
                    # prefetch 4-ko chunks. Chunks alternate across the two
                    # HW DMA queues.
                    nch = 16 if tb == 0 else 4
                    kn = KO // nch
                    for g in range(nch):
                        ksl = slice(g * kn, (g + 1) * kn)
                        # the scalar queue carries ~3MB of weights first, so
                        # X rides sync-only until they drain (tb0/tb1)
                        eng = nc.sync if (tb < 2 or g % 2 == 0) else nc.scalar
                        eng.dma_start(
                            xt[:, ksl],
                            xt_p[ksl, :, tb * 512 : (tb + 1) * 512].rearrange(
                                "ko p t -> p ko t"
                            ),
                        )

                    psq = [
                        psQK.tile([128, 512], f32, tag=f"q{h}", name=f"psq{h}") for h in range(HPC)
                    ]
                    psk = [
                        psQK.tile([128, 512], f32, tag=f"k{h}", name=f"psk{h}") for h in range(HPC)
                    ]
                    # one PSUM bank per V accumulation chain: start=True
                    # clears has_written for the WHOLE bank, so chains must
                    # not share banks
                    psvs = [
                        psV.tile([128, 512], f32, tag=f"v{i}", name=f"psv{i}")
                        for i in range(4)
                    ]
                    # ko-major so each arriving X chunk feeds ~1.3us of PE
                    # work immediately (keeps the PE dense from t~2us).
                    for ko in range(KO):
                        st = ko == 0
                        sp = (ko == KO - 1) and not with_bias
                        for h in range(HPC):
                            nc.tensor.matmul(
                                psq[h],
                                lhsT=wq[:, ko, h * HD : (h + 1) * HD],
                                rhs=xt[:, ko],
                                start=st,
                                stop=sp,
                            )
                        for h in range(HPC):
                            nc.tensor.matmul(
                                psk[h],
                                lhsT=wk[:, ko, h * HD : (h + 1) * HD],
                                rhs=xt[:, ko],
                                start=st,
                                stop=sp,
                            )
                        for tsub in range(4):
                            nc.tensor.matmul(
                                psvs[tsub][:, :256],
                                lhsT=xt[:, ko, tsub * 128 : (tsub + 1) * 128],
                                rhs=wv[:, ko],
                                start=st,
                                stop=sp,
                            )
                    if with_bias:
                        ones512 = masks[0:1, 0, :]  # [1,512] of exact ones
                        for h in range(HPC):
                            nc.tensor.matmul(
                                psq[h],
                                lhsT=biasb[:, 0, h * HD : (h + 1) * HD],
                                rhs=ones512,
                                start=False,
                                stop=True,
                            )
                            nc.tensor.matmul(
                                psk[h],
                                lhsT=biasb[:, 1, h * HD : (h + 1) * HD],
                                rhs=ones512,
                                start=False,
                                stop=True,
                            )
                        for tsub in range(4):
                            nc.tensor.matmul(
                                psvs[tsub][:, :256],
                                lhsT=ones512[:, :128],
                                rhs=biasb[:, 2],
                                start=False,
                                stop=True,
                            )
                    # PSUM->SBUF copies spread across ACT + DVE (Pool
                    # cannot access PSUM)
                    for h in range(HPC):
                        nc.scalar.copy(qt_res[:, b, h, s0 : s0 + 512], psq[h])
                        nc.vector.tensor_copy(kt_res[:, b, h, s0 : s0 + 512], psk[h])
                    for tsub in range(4):
                        nc.vector.tensor_copy(
                            v_res[:, b, sc0 + tsub, :], psvs[tsub][:, :256]
                        )

            # ------------- Phase B + C: attention + out projection -------------
            with (
                tc.tile_pool(name="epool", bufs=34) as epool,
                tc.tile_pool(name="ctxp", bufs=2) as ctxp,
                tc.tile_pool(name="recp", bufs=3) as recp,
                tc.tile_pool(name="obp", bufs=4) as obp,
                tc.tile_pool(name="psS", bufs=2, space="PSUM") as psS,
                tc.tile_pool(name="psC", bufs=2, space="PSUM") as psC,
                tc.tile_pool(name="psD", bufs=2, space="PSUM") as psD,
                tc.tile_pool(name="psO", bufs=2, space="PSUM") as psO,
            ):
                dma_eng = [nc.sync, nc.scalar]

                def emit_outproj_group(b, qb, ctxs, g, cur_nk):
                    qc = 4 * qb + g // 4
                    oc = g % 4
                    pso = psO.tile([128, 512], f32, tag="o", name="pso")
                    for h in range(HPC):
                        nc.tensor.matmul(
                            pso,
                            lhsT=ctxs[h][:, qc * 128 : (qc + 1) * 128],
                            rhs=wot[:, h, oc * 512 : (oc + 1) * 512],
                            start=(h == 0),
                            stop=(h == HPC - 1),
                        )
                    ob = obp.tile([128, 512], bf16, tag="ob", name="ob")
                    # split the PSUM->SBUF casts between DVE and ACT by the
                    # surrounding window's load: in big windows ACT is
                    # exp-bound, in small ones DVE is the tighter engine
                    use_act = (g % 2 == 1) if cur_nk <= 8 else (g % 4 == 3)
                    if use_act:
                        nc.scalar.copy(ob, pso)
                    else:
                        nc.vector.tensor_copy(ob, pso)
                    dma_eng[(qc + oc) % 2].dma_start(
                        out_p[
                            b,
                            qc * 128 : (qc + 1) * 128,
                            oc * 512 : (oc + 1) * 512,
                        ],
                        ob,
                    )

                pending = None
                for b in range(B):
                    ctxs = [
                        ctxp.tile([128, S], bf16, tag=f"ctx{h}", name=f"ctx{h}") for h in range(HPC)
                    ]
                    for qb in range(NQB):
                        nk = 4 * (qb + 1)

                        def cut(t):
                            # diagonal tiles: queries < 128*i are fully
                            # masked for key chunk i -> skip those columns
                            return 128 * (t - 4 * qb) if t >= 4 * qb else 0

                        ess = [[], []]
                        pscs = [
                            psC.tile([128, 512], f32, tag="c", name=f"psc{h}") for h in range(HPC)
                        ]
                        psds = [
                            psD.tile([128, 512], f32, tag="d", name=f"psd{h}") for h in range(HPC)
                        ]
                        # Three interleaved chains at lags 0/1/2: score
                        # matmuls, denominator (ones) matmuls, ctx matmuls.
                        # 6 matmuls per step keep the PE busy past the exp
                        # latency, and the denominator rides the PE without
                        # a separate reduction pass.
                        steps = nk + 2
                        for t in range(steps):
                            # out-projection for the previous window rides
                            # between chain steps so its PSUM->SBUF cast
                            # latency hides behind chain matmuls
                            if pending is not None:
                                g0 = 16 * t // steps
                                g1 = 16 * (t + 1) // steps
                                for g in range(g0, g1):
                                    emit_outproj_group(*pending, g, nk)
                            if t < nk:
                                c0 = cut(t)
                                for h in range(HPC):
                                    pss = psS.tile([128, 512], f32, tag="s")
                                    nc.tensor.matmul(
                                        pss[:, c0:],
                                        lhsT=kt_res[:, b, h, t * 128 : (t + 1) * 128],
                                        rhs=qt_res[
                                            :, b, h,
                                            qb * 512 + c0 : (qb + 1) * 512,
                                        ],
                                        start=True,
                                        stop=True,
                                    )
                                    e = epool.tile([128, 512], bf16, tag="e")
                                    nc.scalar.activation(
                                        e[:, c0:], pss[:, c0:],
                                        mybir.ActivationFunctionType.Exp,
                                        scale=SCALE,
                                    )
                                    if t >= 4 * qb:
                                        # full-width: also zeroes the stale
                                        # (skipped) columns of the ring tile
                                        nc.vector.tensor_mul(
                                            e, e, masks[:, t - 4 * qb]
                                        )
                                    ess[h].append(e)
                            td = t - 1
                            if 0 <= td < nk:
                                c0 = cut(td)
                                for h in range(HPC):
                                    nc.tensor.matmul(
                                        psds[h][:, c0:],
                                        lhsT=onesr,
                                        rhs=ess[h][td][:, c0:],
                                        start=(td == 0),
                                        stop=(td == nk - 1),
                                    )
                            tc_ = t - 2
                            if 0 <= tc_ < nk:
                                c0 = cut(tc_)
                                for h in range(HPC):
                                    nc.tensor.matmul(
                                        pscs[h][:, c0:],
                                        lhsT=v_res[:, b, tc_, h * HD : (h + 1) * HD],
                                        rhs=ess[h][tc_][:, c0:],
                                        start=(tc_ == 0),
                                        stop=(tc_ == nk - 1),
                                    )
                        # --- normalize ---
                        for h in range(HPC):
                            rec = recp.tile([128, 512], f32, tag="rec")
                            nc.vector.reciprocal_approx_fast(rec, psds[h])
                            nc.vector.tensor_mul(
                                ctxs[h][:, qb * 512 : (qb + 1) * 512], pscs[h], rec
                            )
                        pending = (b, qb, ctxs)
                for g in range(16):
                    emit_outproj_group(*pending, g, 16)

    nc.finalize()
    return nc


def _get_nc(with_bias=False):
    if with_bias not in _built:
        _built[with_bias] = _build(with_bias)
    return _built[with_bias]


def kernel(hidden_states, attention_mask, Wq, bq, Wk, bk, Wv, bv, Wo, bo):
    import ml_dtypes

    bf16 = ml_dtypes.bfloat16

    hidden_states = np.asarray(hidden_states, dtype=np.float32)
    Wq, Wk, Wv, Wo = (np.asarray(w, dtype=np.float32) for w in (Wq, Wk, Wv, Wo))
    bq, bk, bv, bo = (np.asarray(v, dtype=np.float32) for v in (bq, bk, bv, bo))

    with_bias = bool(np.any(bq) or np.any(bk) or np.any(bv))

    x = hidden_states.reshape(T, D)
    # [KO, 128, T]: XT[ko, p, t] = x[t, 128*ko + p]
    xt = np.ascontiguousarray(x.T).reshape(KO, 128, T).astype(bf16)

    # causal 0/1 masks for the 4 diagonal-tile offsets: mask[p, i, f] = p + 128*i <= f
    p_idx = np.arange(128)[:, None, None]
    i_idx = np.arange(4)[None, :, None]
    f_idx = np.arange(512)[None, None, :]
    mask = (p_idx + 128 * i_idx <= f_idx).astype(bf16)
    ones = np.ones((128, 128), dtype=bf16)

    in_maps = []
    for c in range(NCORES):
        rows = slice(c * HPC * HD, (c + 1) * HPC * HD)
        wqt = np.ascontiguousarray(Wq[rows, :].T).reshape(KO, 128, HPC * HD)
        wkt = np.ascontiguousarray(Wk[rows, :].T).reshape(KO, 128, HPC * HD)
        wvt = np.ascontiguousarray(Wv[rows, :].T).reshape(KO, 128, HPC * HD)
        # WOT[p, h, n] = Wo[n, c*256 + h*128 + p]
        wot = np.ascontiguousarray(
            Wo[:, rows].T.reshape(HPC, 128, D).transpose(1, 0, 2)
        )
        bias = np.stack([bq[rows], bk[rows], bv[rows], np.zeros(HPC * HD, np.float32)])[
            None
        ]
        in_maps.append(
            {
                "XT": xt,
                "WQT": wqt.astype(bf16),
                "WKT": wkt.astype(bf16),
                "WVT": wvt.astype(bf16),
                "WOT": wot.astype(bf16),
                "BIAS": bias.astype(bf16),
                "MASK": mask,
                "ONES": ones,
            }
        )

    res = run_bass_kernel_spmd(_get_nc(with_bias), in_maps, list(range(NCORES)))
    out = res.results[0]["OUT"].astype(np.float32)
    for c in range(1, NCORES):
        out += res.results[c]["OUT"].astype(np.float32)
    out += bo
    return out


# revision 14
# speedup vs baseline: 1.0693x; 1.0007x over previous
"""Causal multi-head attention on 8 trn2 NeuronCores.

Problem: B=2, S=2048, D=2048, H=16 (HD=128), fp32 in/out.
Sharding: tensor-parallel over heads — core c owns heads {2c, 2c+1} for both
batches. Each core computes its Q/K/V projections, attention for its 4
(batch, head) pairs, and a partial output projection over its head slice.
The host sums the 8 partial outputs and adds the output bias.

All matmul operands are bf16 (1 PE cycle/row, half the DMA/SBUF of fp32),
accumulation in fp32 PSUM. Q^T/K^T/V stay SBUF-resident (no DRAM spill).

Device algorithm (per core):
  Phase A (per 512-token block, ko-streamed): Q^T/K^T (head-dim on
           partitions) and V (tokens on partitions) accumulated over the 16
           contraction chunks; PSUM->SBUF copies spread over ACT/DVE/Pool.
  Phase B (per (b, qb)): S^T score tiles = K^T_chunk.T @ Q^T (keys on
           partitions), E = exp(S^T/sqrt(hd)) on ACT with causal 0/1 mask
           multiplies on diagonal tiles (DVE); the softmax denominator is
           accumulated on the Pool engine (tensor_add of E tiles) and
           reduced with ONE ones-matmul per qb, instead of a PE matmul per
           tile. ctx^T accumulates in PSUM; normalize with DVE reciprocal+
           multiply. No max-subtraction is needed: scores are O(5) so exp
           cannot overflow, and softmax is shift-invariant.
  Phase C: out-projection runs one qb behind phase B (software pipelining
           hides the exp->mask->denom->reciprocal serial chain), partial
           outputs written as bf16, alternating between both HW DMA queues
           (SP and ACT).
"""

import os

import numpy as np

import concourse.bacc as bacc
import concourse.tile as tile
from concourse import mybir
from concourse.bass_utils import run_bass_kernel_spmd


def _install_neff_cache():
    """Cache compiled NEFFs on disk keyed by BIR content hash.

    Purely a compile-time memo: identical BIR -> identical NEFF, so repeat
    runs skip the multi-minute neuronxcc compile. No effect on execution.
    """
    import hashlib
    import shutil

    import concourse.bass2jax as _b2j
    import concourse.bass_utils as _bu

    if getattr(_bu, "_neff_cache_installed", False):
        return
    cache_dir = os.environ.get("NEFF_CACHE_DIR", "/tmp/neff_cache")
    orig = _bu.compile_bir_kernel

    def cached(bir_json, tmpdir, neff_name="file.neff"):
        try:
            os.makedirs(cache_dir, exist_ok=True)
            key = hashlib.sha256(bir_json).hexdigest()[:24]
            cpath = os.path.join(cache_dir, key + ".neff")
            dst = os.path.join(tmpdir, neff_name)
            if os.path.exists(cpath):
                shutil.copy(cpath, dst)
                return dst
            out = orig(bir_json, tmpdir, neff_name)
            shutil.copy(out, cpath)
            return out
        except OSError:
            return orig(bir_json, tmpdir, neff_name)

    _bu.compile_bir_kernel = cached
    _b2j.compile_bir_kernel = cached
    _bu._neff_cache_installed = True


_install_neff_cache()

B, S, D, H = 2, 2048, 2048, 16
HD = D // H          # 128
NCORES = 8
HPC = H // NCORES    # heads per core = 2
T = B * S            # 4096 total token rows
KO = D // 128        # 16 contraction chunks
NTB = T // 512       # 8 phase-A token blocks of 512
NQB = S // 512       # 4 phase-B query blocks per batch
SCALE = 1.0 / float(np.sqrt(HD))

_built = {}


def _build(with_bias):
    f32 = mybir.dt.float32
    bf16 = mybir.dt.bfloat16
    f32r = mybir.dt.float32r

    nc = bacc.Bacc(None, target_bir_lowering=False)

    # ---- per-core DRAM parameters (host supplies per-core shards) ----
    xt_p = nc.declare_dram_parameter("XT", [KO, 128, T], bf16, False)
    wqt_p = nc.declare_dram_parameter("WQT", [KO, 128, HPC * HD], bf16, False)
    wkt_p = nc.declare_dram_parameter("WKT", [KO, 128, HPC * HD], bf16, False)
    wvt_p = nc.declare_dram_parameter("WVT", [KO, 128, HPC * HD], bf16, False)
    wot_p = nc.declare_dram_parameter("WOT", [128, HPC, D], bf16, False)
    bias_p = nc.declare_dram_parameter("BIAS", [1, 4, HPC * HD], bf16, False)
    mask_p = nc.declare_dram_parameter("MASK", [128, 4, 512], bf16, False)
    ones_p = nc.declare_dram_parameter("ONES", [128, 128], bf16, False)
    out_p = nc.declare_dram_parameter("OUT", [B, S, D], bf16, True)

    with tile.TileContext(nc) as tc:
        with tc.tile_pool(name="persist", bufs=1) as persist:
            qt_res = persist.tile([128, B, HPC, S], bf16)
            kt_res = persist.tile([128, B, HPC, S], bf16)
            # V natural layout: v_res[p, b, sc, h*HD+d] = V[b, 128*sc+p, h, d]
            v_res = persist.tile([128, B, S // 128, HPC * HD], bf16)
            wot = persist.tile([128, HPC, D], bf16)
            masks = persist.tile([128, 4, 512], bf16)
            onesr = persist.tile([128, 128], bf16)
            if with_bias:
                biasb = persist.tile([1, 4, HPC * HD], bf16)

            # ---------------- Phase A: projections ----------------
            with (
                tc.tile_pool(name="wqkv", bufs=1) as wpool,
                tc.tile_pool(name="xs", bufs=2) as xpool,
                tc.tile_pool(name="psQK", bufs=1, space="PSUM") as psQK,
                tc.tile_pool(name="psV", bufs=1, space="PSUM") as psV,
            ):
                wq = wpool.tile([128, KO, HPC * HD], bf16, tag="wq")
                wk = wpool.tile([128, KO, HPC * HD], bf16, tag="wk")
                wv = wpool.tile([128, KO, HPC * HD], bf16, tag="wv")
                # weight/constant loads ride the ACT hw queue; X rides
                # both queues. Interleave wq/wk/wv chunks so the ko-major
                # matmul stream (needs all three per ko) starts earliest.
                for g in range(4):
                    ksl = slice(g * 4, (g + 1) * 4)
                    for wt, wp in ((wq, wqt_p), (wk, wkt_p), (wv, wvt_p)):
                        nc.scalar.dma_start(
                            wt[:, ksl], wp[ksl].rearrange("ko p m -> p ko m")
                        )
                if with_bias:
                    nc.scalar.dma_start(biasb, bias_p[:])
                nc.scalar.dma_start(masks, mask_p[:])
                nc.scalar.dma_start(onesr, ones_p[:])
                nc.scalar.dma_start(wot, wot_p[:])
                warm = persist.tile([1, 8], f32)
                nc.vector.memset(warm, 1.0)
                nc.scalar.activation(
                    warm, warm, mybir.ActivationFunctionType.Exp
                )

                for tb in range(NTB):
                    b = (tb * 512) // S
                    s0 = (tb * 512) % S
                    sc0 = s0 // 128
                    xt = xpool.tile([128, KO, 512], bf16, tag="xt")
                    # tb0 streams 1-ko chunks (matches the 6-matmul/ko
                    # consumption rate at queue-ramp bandwidth); later tbs
                    # prefetch 4-ko chunks. Chunks alternate across the two
                    # HW DMA queues.
                    nch = 16 if tb == 0 else 4
                    kn = KO // nch
                    for g in range(nch):
                        ksl = slice(g * kn, (g + 1) * kn)
                        # the scalar queue carries ~3MB of weights first, so
                        # X rides sync-only until they drain (tb0/tb1)
                        eng = nc.sync if (tb < 2 or g % 2 == 0) else nc.scalar
                        eng.dma_start(
                            xt[:, ksl],
                            xt_p[ksl, :, tb * 512 : (tb + 1) * 512].rearrange(
                                "ko p t -> p ko t"
                            ),
                        )

                    psq = [
                        psQK.tile([128, 512], f32, tag=f"q{h}", name=f"psq{h}") for h in range(HPC)
                    ]
                    psk = [
                        psQK.tile([128, 512], f32, tag=f"k{h}", name=f"psk{h}") for h in range(HPC)
                    ]
                    # one PSUM bank per V accumulation chain: start=True
                    # clears has_written for the WHOLE bank, so chains must
                    # not share banks
                    psvs = [
                        psV.tile([128, 512], f32, tag=f"v{i}", name=f"psv{i}")
                        for i in range(4)
                    ]
                    # ko-major so each arriving X chunk feeds ~1.3us of PE
                    # work immediately (keeps the PE dense from t~2us).
                    for ko in range(KO):
                        st = ko == 0
                        sp = (ko == KO - 1) and not with_bias
                        for h in range(HPC):
                            nc.tensor.matmul(
                                psq[h],
                                lhsT=wq[:, ko, h * HD : (h + 1) * HD],
                                rhs=xt[:, ko],
                                start=st,
                                stop=sp,
                            )
                        for h in range(HPC):
                            nc.tensor.matmul(
                                psk[h],
                                lhsT=wk[:, ko, h * HD : (h + 1) * HD],
                                rhs=xt[:, ko],
                                start=st,
                                stop=sp,
                            )
                        for tsub in range(4):
                            nc.tensor.matmul(
                                psvs[tsub][:, :256],
                                lhsT=xt[:, ko, tsub * 128 : (tsub + 1) * 128],
                                rhs=wv[:, ko],
                                start=st,
                                stop=sp,
                            )
                    if with_bias:
                        ones512 = masks[0:1, 0, :]  # [1,512] of exact ones
                        for h in range(HPC):
                            nc.tensor.matmul(
                                psq[h],
                                lhsT=biasb[:, 0, h * HD : (h + 1) * HD],
                                rhs=ones512,
                                start=False,
                                stop=True,
                            )
                            nc.tensor.matmul(
                                psk[h],
                                lhsT=biasb[:, 1, h * HD : (h + 1) * HD],
                                rhs=ones512,
                                start=False,
                                stop=True,
                            )
                        for tsub in range(4):
                            nc.tensor.matmul(
                                psvs[tsub][:, :256],
                                lhsT=ones512[:, :128],
                                rhs=biasb[:, 2],
                                start=False,
                                stop=True,
                            )
                    # PSUM->SBUF copies spread across ACT + DVE (Pool
                    # cannot access PSUM)
                    for h in range(HPC):
                        nc.scalar.copy(qt_res[:, b, h, s0 : s0 + 512], psq[h])
                        nc.vector.tensor_copy(kt_res[:, b, h, s0 : s0 + 512], psk[h])
                    for tsub in range(4):
                        nc.vector.tensor_copy(
                            v_res[:, b, sc0 + tsub, :], psvs[tsub][:, :256]
                        )

            # ------------- Phase B + C: attention + out projection -------------
            with (
                tc.tile_pool(name="epool", bufs=34) as epool,
                tc.tile_pool(name="ctxp", bufs=2) as ctxp,
                tc.tile_pool(name="recp", bufs=3) as recp,
                tc.tile_pool(name="obp", bufs=4) as obp,
                tc.tile_pool(name="psS", bufs=2, space="PSUM") as psS,
                tc.tile_pool(name="psC", bufs=2, space="PSUM") as psC,
                tc.tile_pool(name="psD", bufs=2, space="PSUM") as psD,
                tc.tile_pool(name="psO", bufs=2, space="PSUM") as psO,
            ):
                dma_eng = [nc.sync, nc.scalar]

                def emit_outproj_group(b, qb, ctxs, g, cur_nk):
                    qc = 4 * qb + g // 4
                    oc = g % 4
                    pso = psO.tile([128, 512], f32, tag="o", name="pso")
                    for h in range(HPC):
                        nc.tensor.matmul(
                            pso,
                            lhsT=ctxs[h][:, qc * 128 : (qc + 1) * 128],
                            rhs=wot[:, h, oc * 512 : (oc + 1) * 512],
                            start=(h == 0),
                            stop=(h == HPC - 1),
                        )
                    ob = obp.tile([128, 512], bf16, tag="ob", name="ob")
                    # split the PSUM->SBUF casts between DVE and ACT by the
                    # surrounding window's load: in big windows ACT is
                    # exp-bound, in small ones DVE is the tighter engine
                    use_act = (g % 2 == 1) if cur_nk <= 8 else (g % 4 == 3)
                    if use_act:
                        nc.scalar.copy(ob, pso)
                    else:
                        nc.vector.tensor_copy(ob, pso)
                    dma_eng[(qc + oc) % 2].dma_start(
                        out_p[
                            b,
                            qc * 128 : (qc + 1) * 128,
                            oc * 512 : (oc + 1) * 512,
                        ],
                        ob,
                    )

                pending = None
                for b in range(B):
                    ctxs = [
                        ctxp.tile([128, S], bf16, tag=f"ctx{h}", name=f"ctx{h}") for h in range(HPC)
                    ]
                    for qb in range(NQB):
                        nk = 4 * (qb + 1)

                        def cut(t):
                            # diagonal tiles: queries < 128*i are fully
                            # masked for key chunk i -> skip those columns
                            return 128 * (t - 4 * qb) if t >= 4 * qb else 0

                        ess = [[], []]
                        pscs = [
                            psC.tile([128, 512], f32, tag="c", name=f"psc{h}") for h in range(HPC)
                        ]
                        psds = [
                            psD.tile([128, 512], f32, tag="d", name=f"psd{h}") for h in range(HPC)
                        ]
                        # Three interleaved chains at lags 0/1/2: score
                        # matmuls, denominator (ones) matmuls, ctx matmuls.
                        # 6 matmuls per step keep the PE busy past the exp
                        # latency, and the denominator rides the PE without
                        # a separate reduction pass.
                        steps = nk + 2
                        for t in range(steps):
                            # out-projection for the previous window rides
                            # between chain steps so its PSUM->SBUF cast
                            # latency hides behind chain matmuls
                            if pending is not None:
                                g0 = 16 * t // steps
                                g1 = 16 * (t + 1) // steps
                                for g in range(g0, g1):
                                    emit_outproj_group(*pending, g, nk)
                            if t < nk:
                                c0 = cut(t)
                                for h in range(HPC):
                                    pss = psS.tile([128, 512], f32, tag="s")
                                    nc.tensor.matmul(
                                        pss[:, c0:],
                                        lhsT=kt_res[:, b, h, t * 128 : (t + 1) * 128],
                                        rhs=qt_res[
                                            :, b, h,
                                            qb * 512 + c0 : (qb + 1) * 512,
                                        ],
                                        start=True,
                                        stop=True,
                                    )
                                    e = epool.tile([128, 512], bf16, tag="e")
                                    nc.scalar.activation(
                                        e[:, c0:], pss[:, c0:],
                                        mybir.ActivationFunctionType.Exp,
                                        scale=SCALE,
                                    )
                                    if t >= 4 * qb:
                                        # full-width: also zeroes the stale
                                        # (skipped) columns of the ring tile
                                        nc.vector.tensor_mul(
                                            e, e, masks[:, t - 4 * qb]
                                        )
                                    ess[h].append(e)
                            td = t - 1
                            if 0 <= td < nk:
                                c0 = cut(td)
                                for h in range(HPC):
                                    nc.tensor.matmul(
                                        psds[h][:, c0:],
                                        lhsT=onesr,
                                        rhs=ess[h][td][:, c0:],
                                        start=(td == 0),
                                        stop=(td == nk - 1),
                                    )
                            tc_ = t - 2
                            if 0 <= tc_ < nk:
                                c0 = cut(tc_)
                                for h in range(HPC):
                                    nc.tensor.matmul(
                                        pscs[h][:, c0:],
                                        lhsT=v_res[:, b, tc_, h * HD : (h + 1) * HD],
                                        rhs=ess[h][tc_][:, c0:],
                                        start=(tc_ == 0),
                                        stop=(tc_ == nk - 1),
                                    )
                        is_last = b == B - 1 and qb == NQB - 1
                        if is_last:
                            last_ps = (pscs, psds)
                        else:
                            # --- normalize ---
                            for h in range(HPC):
                                rec = recp.tile([128, 512], f32, tag="rec")
                                nc.vector.reciprocal_approx_fast(rec, psds[h])
                                nc.vector.tensor_mul(
                                    ctxs[h][:, qb * 512 : (qb + 1) * 512],
                                    pscs[h], rec,
                                )
                        pending = (b, qb, ctxs)
                # final window: normalize in 128-column chunks so its own
                # out-projection can start while later chunks still divide
                pscs, psds = last_ps
                b, qb, ctxs = pending
                recs = []
                for h in range(HPC):
                    rec = recp.tile([128, 512], f32, tag="rec")
                    nc.vector.reciprocal_approx_fast(rec, psds[h])
                    recs.append(rec)
                for cc in range(4):
                    csl = slice(cc * 128, (cc + 1) * 128)
                    osl = slice(qb * 512 + cc * 128, qb * 512 + (cc + 1) * 128)
                    for h in range(HPC):
                        nc.vector.tensor_mul(
                            ctxs[h][:, osl], pscs[h][:, csl], recs[h][:, csl]
                        )
                    for g in range(4 * cc, 4 * cc + 4):
                        emit_outproj_group(b, qb, ctxs, g, 16)

    nc.finalize()
    return nc


def _get_nc(with_bias=False):
    if with_bias not in _built:
        _built[with_bias] = _build(with_bias)
    return _built[with_bias]


def kernel(hidden_states, attention_mask, Wq, bq, Wk, bk, Wv, bv, Wo, bo):
    import ml_dtypes

    bf16 = ml_dtypes.bfloat16

    hidden_states = np.asarray(hidden_states, dtype=np.float32)
    Wq, Wk, Wv, Wo = (np.asarray(w, dtype=np.float32) for w in (Wq, Wk, Wv, Wo))
    bq, bk, bv, bo = (np.asarray(v, dtype=np.float32) for v in (bq, bk, bv, bo))

    with_bias = bool(np.any(bq) or np.any(bk) or np.any(bv))

    x = hidden_states.reshape(T, D)
    # [KO, 128, T]: XT[ko, p, t] = x[t, 128*ko + p]
    xt = np.ascontiguousarray(x.T).reshape(KO, 128, T).astype(bf16)

    # causal 0/1 masks for the 4 diagonal-tile offsets: mask[p, i, f] = p + 128*i <= f
    p_idx = np.arange(128)[:, None, None]
    i_idx = np.arange(4)[None, :, None]
    f_idx = np.arange(512)[None, None, :]
    mask = (p_idx + 128 * i_idx <= f_idx).astype(bf16)
    ones = np.ones((128, 128), dtype=bf16)

    in_maps = []
    for c in range(NCORES):
        rows = slice(c * HPC * HD, (c + 1) * HPC * HD)
        wqt = np.ascontiguousarray(Wq[rows, :].T).reshape(KO, 128, HPC * HD)
        wkt = np.ascontiguousarray(Wk[rows, :].T).reshape(KO, 128, HPC * HD)
        wvt = np.ascontiguousarray(Wv[rows, :].T).reshape(KO, 128, HPC * HD)
        # WOT[p, h, n] = Wo[n, c*256 + h*128 + p]
        wot = np.ascontiguousarray(
            Wo[:, rows].T.reshape(HPC, 128, D).transpose(1, 0, 2)
        )
        bias = np.stack([bq[rows], bk[rows], bv[rows], np.zeros(HPC * HD, np.float32)])[
            None
        ]
        in_maps.append(
            {
                "XT": xt,
                "WQT": wqt.astype(bf16),
                "WKT": wkt.astype(bf16),
                "WVT": wvt.astype(bf16),
                "WOT": wot.astype(bf16),
                "BIAS": bias.astype(bf16),
                "MASK": mask,
                "ONES": ones,
            }
        )

    res = run_bass_kernel_spmd(_get_nc(with_bias), in_maps, list(range(NCORES)))
    out = res.results[0]["OUT"].astype(np.float32)
    for c in range(1, NCORES):
        out += res.results[c]["OUT"].astype(np.float32)
    out += bo
    return out


# revision 15
# speedup vs baseline: 1.0695x; 1.0002x over previous
"""Causal multi-head attention on 8 trn2 NeuronCores.

Problem: B=2, S=2048, D=2048, H=16 (HD=128), fp32 in/out.
Sharding: tensor-parallel over heads — core c owns heads {2c, 2c+1} for both
batches. Each core computes its Q/K/V projections, attention for its 4
(batch, head) pairs, and a partial output projection over its head slice.
The host sums the 8 partial outputs and adds the output bias.

All matmul operands are bf16 (1 PE cycle/row, half the DMA/SBUF of fp32),
accumulation in fp32 PSUM. Q^T/K^T/V stay SBUF-resident (no DRAM spill).

Device algorithm (per core):
  Phase A (per 512-token block, ko-streamed): Q^T/K^T (head-dim on
           partitions) and V (tokens on partitions) accumulated over the 16
           contraction chunks; PSUM->SBUF copies spread over ACT/DVE/Pool.
  Phase B (per (b, qb)): S^T score tiles = K^T_chunk.T @ Q^T (keys on
           partitions), E = exp(S^T/sqrt(hd)) on ACT with causal 0/1 mask
           multiplies on diagonal tiles (DVE); the softmax denominator is
           accumulated on the Pool engine (tensor_add of E tiles) and
           reduced with ONE ones-matmul per qb, instead of a PE matmul per
           tile. ctx^T accumulates in PSUM; normalize with DVE reciprocal+
           multiply. No max-subtraction is needed: scores are O(5) so exp
           cannot overflow, and softmax is shift-invariant.
  Phase C: out-projection runs one qb behind phase B (software pipelining
           hides the exp->mask->denom->reciprocal serial chain), partial
           outputs written as bf16, alternating between both HW DMA queues
           (SP and ACT).
"""

import os

import numpy as np

import concourse.bacc as bacc
import concourse.tile as tile
from concourse import mybir
from concourse.bass_utils import run_bass_kernel_spmd


def _install_neff_cache():
    """Cache compiled NEFFs on disk keyed by BIR content hash.

    Purely a compile-time memo: identical BIR -> identical NEFF, so repeat
    runs skip the multi-minute neuronxcc compile. No effect on execution.
    """
    import hashlib
    import shutil

    import concourse.bass2jax as _b2j
    import concourse.bass_utils as _bu

    if getattr(_bu, "_neff_cache_installed", False):
        return
    cache_dir = os.environ.get("NEFF_CACHE_DIR", "/tmp/neff_cache")
    orig = _bu.compile_bir_kernel

    def cached(bir_json, tmpdir, neff_name="file.neff"):
        try:
            os.makedirs(cache_dir, exist_ok=True)
            key = hashlib.sha256(bir_json).hexdigest()[:24]
            cpath = os.path.join(cache_dir, key + ".neff")
            dst = os.path.join(tmpdir, neff_name)
            if os.path.exists(cpath):
                shutil.copy(cpath, dst)
                return dst
            out = orig(bir_json, tmpdir, neff_name)
            shutil.copy(out, cpath)
            return out
        except OSError:
            return orig(bir_json, tmpdir, neff_name)

    _bu.compile_bir_kernel = cached
    _b2j.compile_bir_kernel = cached
    _bu._neff_cache_installed = True


_install_neff_cache()

B, S, D, H = 2, 2048, 2048, 16
HD = D // H          # 128
NCORES = 8
HPC = H // NCORES    # heads per core = 2
T = B * S            # 4096 total token rows
KO = D // 128        # 16 contraction chunks
NTB = T // 512       # 8 phase-A token blocks of 512
NQB = S // 512       # 4 phase-B query blocks per batch
SCALE = 1.0 / float(np.sqrt(HD))

_built = {}


def _build(with_bias):
    f32 = mybir.dt.float32
    bf16 = mybir.dt.bfloat16
    f32r = mybir.dt.float32r

    nc = bacc.Bacc(None, target_bir_lowering=False)

    # ---- per-core DRAM parameters (host supplies per-core shards) ----
    xt_p = nc.declare_dram_parameter("XT", [KO, 128, T], bf16, False)
    wqt_p = nc.declare_dram_parameter("WQT", [KO, 128, HPC * HD], bf16, False)
    wkt_p = nc.declare_dram_parameter("WKT", [KO, 128, HPC * HD], bf16, False)
    wvt_p = nc.declare_dram_parameter("WVT", [KO, 128, HPC * HD], bf16, False)
    wot_p = nc.declare_dram_parameter("WOT", [128, HPC, D], bf16, False)
    bias_p = nc.declare_dram_parameter("BIAS", [1, 4, HPC * HD], bf16, False)
    mask_p = nc.declare_dram_parameter("MASK", [128, 4, 512], bf16, False)
    ones_p = nc.declare_dram_parameter("ONES", [128, 128], bf16, False)
    out_p = nc.declare_dram_parameter("OUT", [B, S, D], bf16, True)

    with tile.TileContext(nc) as tc:
        with tc.tile_pool(name="persist", bufs=1) as persist:
            qt_res = persist.tile([128, B, HPC, S], bf16)
            kt_res = persist.tile([128, B, HPC, S], bf16)
            # V natural layout: v_res[p, b, sc, h*HD+d] = V[b, 128*sc+p, h, d]
            v_res = persist.tile([128, B, S // 128, HPC * HD], bf16)
            wot = persist.tile([128, HPC, D], bf16)
            masks = persist.tile([128, 4, 512], bf16)
            onesr = persist.tile([128, 128], bf16)
            if with_bias:
                biasb = persist.tile([1, 4, HPC * HD], bf16)

            # ---------------- Phase A: projections ----------------
            with (
                tc.tile_pool(name="wqkv", bufs=1) as wpool,
                tc.tile_pool(name="xs", bufs=2) as xpool,
                tc.tile_pool(name="psQK", bufs=1, space="PSUM") as psQK,
                tc.tile_pool(name="psV", bufs=1, space="PSUM") as psV,
            ):
                wq = wpool.tile([128, KO, HPC * HD], bf16, tag="wq")
                wk = wpool.tile([128, KO, HPC * HD], bf16, tag="wk")
                wv = wpool.tile([128, KO, HPC * HD], bf16, tag="wv")
                # weight/constant loads ride the ACT hw queue; X rides
                # both queues. Interleave wq/wk/wv chunks so the ko-major
                # matmul stream (needs all three per ko) starts earliest.
                for g in range(4):
                    ksl = slice(g * 4, (g + 1) * 4)
                    for wt, wp in ((wq, wqt_p), (wk, wkt_p), (wv, wvt_p)):
                        nc.scalar.dma_start(
                            wt[:, ksl], wp[ksl].rearrange("ko p m -> p ko m")
                        )
                if with_bias:
                    nc.scalar.dma_start(biasb, bias_p[:])
                nc.scalar.dma_start(masks, mask_p[:])
                nc.scalar.dma_start(onesr, ones_p[:])
                nc.scalar.dma_start(wot, wot_p[:])
                warm = persist.tile([1, 8], f32)
                nc.vector.memset(warm, 1.0)
                nc.scalar.activation(
                    warm, warm, mybir.ActivationFunctionType.Exp
                )

                for tb in range(NTB):
                    b = (tb * 512) // S
                    s0 = (tb * 512) % S
                    sc0 = s0 // 128
                    xt = xpool.tile([128, KO, 512], bf16, tag="xt")
                    # tb0 streams 1-ko chunks (matches the 6-matmul/ko
                    # consumption rate at queue-ramp bandwidth); later tbs
                    # prefetch 4-ko chunks. Chunks alternate across the two
                    # HW DMA queues.
                    nch = 16 if tb == 0 else 4
                    kn = KO // nch
                    for g in range(nch):
                        ksl = slice(g * kn, (g + 1) * kn)
                        # the scalar queue carries ~3MB of weights first, so
                        # X rides sync-only until they drain (tb0/tb1)
                        eng = nc.sync if (tb < 2 or g % 2 == 0) else nc.scalar
                        eng.dma_start(
                            xt[:, ksl],
                            xt_p[ksl, :, tb * 512 : (tb + 1) * 512].rearrange(
                                "ko p t -> p ko t"
                            ),
                        )

                    psq = [
                        psQK.tile([128, 512], f32, tag=f"q{h}", name=f"psq{h}") for h in range(HPC)
                    ]
                    psk = [
                        psQK.tile([128, 512], f32, tag=f"k{h}", name=f"psk{h}") for h in range(HPC)
                    ]
                    # one PSUM bank per V accumulation chain: start=True
                    # clears has_written for the WHOLE bank, so chains must
                    # not share banks
                    psvs = [
                        psV.tile([128, 512], f32, tag=f"v{i}", name=f"psv{i}")
                        for i in range(4)
                    ]
                    # ko-major so each arriving X chunk feeds ~1.3us of PE
                    # work immediately (keeps the PE dense from t~2us).
                    for ko in range(KO):
                        st = ko == 0
                        sp = (ko == KO - 1) and not with_bias
                        for h in range(HPC):
                            nc.tensor.matmul(
                                psq[h],
                                lhsT=wq[:, ko, h * HD : (h + 1) * HD],
                                rhs=xt[:, ko],
                                start=st,
                                stop=sp,
                            )
                        for h in range(HPC):
                            nc.tensor.matmul(
                                psk[h],
                                lhsT=wk[:, ko, h * HD : (h + 1) * HD],
                                rhs=xt[:, ko],
                                start=st,
                                stop=sp,
                            )
                        for tsub in range(4):
                            nc.tensor.matmul(
                                psvs[tsub][:, :256],
                                lhsT=xt[:, ko, tsub * 128 : (tsub + 1) * 128],
                                rhs=wv[:, ko],
                                start=st,
                                stop=sp,
                            )
                    if with_bias:
                        ones512 = masks[0:1, 0, :]  # [1,512] of exact ones
                        for h in range(HPC):
                            nc.tensor.matmul(
                                psq[h],
                                lhsT=biasb[:, 0, h * HD : (h + 1) * HD],
                                rhs=ones512,
                                start=False,
                                stop=True,
                            )
                            nc.tensor.matmul(
                                psk[h],
                                lhsT=biasb[:, 1, h * HD : (h + 1) * HD],
                                rhs=ones512,
                                start=False,
                                stop=True,
                            )
                        for tsub in range(4):
                            nc.tensor.matmul(
                                psvs[tsub][:, :256],
                                lhsT=ones512[:, :128],
                                rhs=biasb[:, 2],
                                start=False,
                                stop=True,
                            )
                    # PSUM->SBUF copies spread across ACT + DVE (Pool
                    # cannot access PSUM)
                    for h in range(HPC):
                        nc.scalar.copy(qt_res[:, b, h, s0 : s0 + 512], psq[h])
                        nc.vector.tensor_copy(kt_res[:, b, h, s0 : s0 + 512], psk[h])
                    for tsub in range(4):
                        nc.vector.tensor_copy(
                            v_res[:, b, sc0 + tsub, :], psvs[tsub][:, :256]
                        )

            # ------------- Phase B + C: attention + out projection -------------
            with (
                tc.tile_pool(name="epool", bufs=34) as epool,
                tc.tile_pool(name="ctxp", bufs=2) as ctxp,
                tc.tile_pool(name="recp", bufs=3) as recp,
                tc.tile_pool(name="obp", bufs=4) as obp,
                tc.tile_pool(name="psS", bufs=2, space="PSUM") as psS,
                tc.tile_pool(name="psC", bufs=2, space="PSUM") as psC,
                tc.tile_pool(name="psD", bufs=2, space="PSUM") as psD,
                tc.tile_pool(name="psO", bufs=2, space="PSUM") as psO,
            ):
                dma_eng = [nc.sync, nc.scalar]

                def emit_outproj_group(b, qb, ctxs, g, cur_nk):
                    qc = 4 * qb + g // 4
                    oc = g % 4
                    pso = psO.tile([128, 512], f32, tag="o", name="pso")
                    for h in range(HPC):
                        nc.tensor.matmul(
                            pso,
                            lhsT=ctxs[h][:, qc * 128 : (qc + 1) * 128],
                            rhs=wot[:, h, oc * 512 : (oc + 1) * 512],
                            start=(h == 0),
                            stop=(h == HPC - 1),
                        )
                    ob = obp.tile([128, 512], bf16, tag="ob", name="ob")
                    # split the PSUM->SBUF casts between DVE and ACT by the
                    # surrounding window's load: in big windows ACT is
                    # exp-bound; in the exp-free tail (cur_nk=0) split evenly
                    if cur_nk == 0:
                        use_act = g % 2 == 1
                    elif cur_nk <= 8:
                        use_act = g % 3 == 2
                    else:
                        use_act = g % 4 == 3
                    if use_act:
                        nc.scalar.copy(ob, pso)
                    else:
                        nc.vector.tensor_copy(ob, pso)
                    dma_eng[(qc + oc) % 2].dma_start(
                        out_p[
                            b,
                            qc * 128 : (qc + 1) * 128,
                            oc * 512 : (oc + 1) * 512,
                        ],
                        ob,
                    )

                pending = None
                for b in range(B):
                    ctxs = [
                        ctxp.tile([128, S], bf16, tag=f"ctx{h}", name=f"ctx{h}") for h in range(HPC)
                    ]
                    for qb in range(NQB):
                        nk = 4 * (qb + 1)

                        def cut(t):
                            # diagonal tiles: queries < 128*i are fully
                            # masked for key chunk i -> skip those columns
                            return 128 * (t - 4 * qb) if t >= 4 * qb else 0

                        ess = [[], []]
                        pscs = [
                            psC.tile([128, 512], f32, tag="c", name=f"psc{h}") for h in range(HPC)
                        ]
                        psds = [
                            psD.tile([128, 512], f32, tag="d", name=f"psd{h}") for h in range(HPC)
                        ]
                        # Three interleaved chains at lags 0/1/2: score
                        # matmuls, denominator (ones) matmuls, ctx matmuls.
                        # 6 matmuls per step keep the PE busy past the exp
                        # latency, and the denominator rides the PE without
                        # a separate reduction pass.
                        steps = nk + 2
                        for t in range(steps):
                            # out-projection for the previous window rides
                            # between chain steps so its PSUM->SBUF cast
                            # latency hides behind chain matmuls
                            if pending is not None:
                                g0 = 16 * t // steps
                                g1 = 16 * (t + 1) // steps
                                for g in range(g0, g1):
                                    emit_outproj_group(*pending, g, nk)
                            if t < nk:
                                c0 = cut(t)
                                for h in range(HPC):
                                    pss = psS.tile([128, 512], f32, tag="s")
                                    nc.tensor.matmul(
                                        pss[:, c0:],
                                        lhsT=kt_res[:, b, h, t * 128 : (t + 1) * 128],
                                        rhs=qt_res[
                                            :, b, h,
                                            qb * 512 + c0 : (qb + 1) * 512,
                                        ],
                                        start=True,
                                        stop=True,
                                    )
                                    e = epool.tile([128, 512], bf16, tag="e")
                                    nc.scalar.activation(
                                        e[:, c0:], pss[:, c0:],
                                        mybir.ActivationFunctionType.Exp,
                                        scale=SCALE,
                                    )
                                    if t >= 4 * qb:
                                        # full-width: also zeroes the stale
                                        # (skipped) columns of the ring tile
                                        nc.vector.tensor_mul(
                                            e, e, masks[:, t - 4 * qb]
                                        )
                                    ess[h].append(e)
                            td = t - 1
                            if 0 <= td < nk:
                                c0 = cut(td)
                                for h in range(HPC):
                                    nc.tensor.matmul(
                                        psds[h][:, c0:],
                                        lhsT=onesr,
                                        rhs=ess[h][td][:, c0:],
                                        start=(td == 0),
                                        stop=(td == nk - 1),
                                    )
                            tc_ = t - 2
                            if 0 <= tc_ < nk:
                                c0 = cut(tc_)
                                for h in range(HPC):
                                    nc.tensor.matmul(
                                        pscs[h][:, c0:],
                                        lhsT=v_res[:, b, tc_, h * HD : (h + 1) * HD],
                                        rhs=ess[h][tc_][:, c0:],
                                        start=(tc_ == 0),
                                        stop=(tc_ == nk - 1),
                                    )
                        is_last = b == B - 1 and qb == NQB - 1
                        if is_last:
                            last_ps = (pscs, psds)
                        else:
                            # --- normalize ---
                            for h in range(HPC):
                                rec = recp.tile([128, 512], f32, tag="rec")
                                nc.vector.reciprocal_approx_fast(rec, psds[h])
                                nc.vector.tensor_mul(
                                    ctxs[h][:, qb * 512 : (qb + 1) * 512],
                                    pscs[h], rec,
                                )
                        pending = (b, qb, ctxs)
                # final window: normalize in 128-column chunks so its own
                # out-projection can start while later chunks still divide
                pscs, psds = last_ps
                b, qb, ctxs = pending
                recs = []
                for h in range(HPC):
                    rec = recp.tile([128, 512], f32, tag="rec")
                    nc.vector.reciprocal_approx_fast(rec, psds[h])
                    recs.append(rec)
                for cc in range(4):
                    csl = slice(cc * 128, (cc + 1) * 128)
                    osl = slice(qb * 512 + cc * 128, qb * 512 + (cc + 1) * 128)
                    for h in range(HPC):
                        nc.vector.tensor_mul(
                            ctxs[h][:, osl], pscs[h][:, csl], recs[h][:, csl]
                        )
                    for g in range(4 * cc, 4 * cc + 4):
                        emit_outproj_group(b, qb, ctxs, g, 0)

    nc.finalize()
    return nc


def _get_nc(with_bias=False):
    if with_bias not in _built:
        _built[with_bias] = _build(with_bias)
    return _built[with_bias]


def kernel(hidden_states, attention_mask, Wq, bq, Wk, bk, Wv, bv, Wo, bo):
    import ml_dtypes

    bf16 = ml_dtypes.bfloat16

    hidden_states = np.asarray(hidden_states, dtype=np.float32)
    Wq, Wk, Wv, Wo = (np.asarray(w, dtype=np.float32) for w in (Wq, Wk, Wv, Wo))
    bq, bk, bv, bo = (np.asarray(v, dtype=np.float32) for v in (bq, bk, bv, bo))

    with_bias = bool(np.any(bq) or np.any(bk) or np.any(bv))

    x = hidden_states.reshape(T, D)
    # [KO, 128, T]: XT[ko, p, t] = x[t, 128*ko + p]
    xt = np.ascontiguousarray(x.T).reshape(KO, 128, T).astype(bf16)

    # causal 0/1 masks for the 4 diagonal-tile offsets: mask[p, i, f] = p + 128*i <= f
    p_idx = np.arange(128)[:, None, None]
    i_idx = np.arange(4)[None, :, None]
    f_idx = np.arange(512)[None, None, :]
    mask = (p_idx + 128 * i_idx <= f_idx).astype(bf16)
    ones = np.ones((128, 128), dtype=bf16)

    in_maps = []
    for c in range(NCORES):
        rows = slice(c * HPC * HD, (c + 1) * HPC * HD)
        wqt = np.ascontiguousarray(Wq[rows, :].T).reshape(KO, 128, HPC * HD)
        wkt = np.ascontiguousarray(Wk[rows, :].T).reshape(KO, 128, HPC * HD)
        wvt = np.ascontiguousarray(Wv[rows, :].T).reshape(KO, 128, HPC * HD)
        # WOT[p, h, n] = Wo[n, c*256 + h*128 + p]
        wot = np.ascontiguousarray(
            Wo[:, rows].T.reshape(HPC, 128, D).transpose(1, 0, 2)
        )
        bias = np.stack([bq[rows], bk[rows], bv[rows], np.zeros(HPC * HD, np.float32)])[
            None
        ]
        in_maps.append(
            {
                "XT": xt,
                "WQT": wqt.astype(bf16),
                "WKT": wkt.astype(bf16),
                "WVT": wvt.astype(bf16),
                "WOT": wot.astype(bf16),
                "BIAS": bias.astype(bf16),
                "MASK": mask,
                "ONES": ones,
            }
        )

    res = run_bass_kernel_spmd(_get_nc(with_bias), in_maps, list(range(NCORES)))
    out = res.results[0]["OUT"].astype(np.float32)
    for c in range(1, NCORES):
        out += res.results[c]["OUT"].astype(np.float32)
    out += bo
    return out


# revision 17
# speedup vs baseline: 1.0707x; 1.0011x over previous
"""Causal multi-head attention on 8 trn2 NeuronCores.

Problem: B=2, S=2048, D=2048, H=16 (HD=128), fp32 in/out.
Sharding: tensor-parallel over heads — core c owns heads {2c, 2c+1} for both
batches. Each core computes its Q/K/V projections, attention for its 4
(batch, head) pairs, and a partial output projection over its head slice.
The host sums the 8 partial outputs and adds the output bias.

All matmul operands are bf16 (1 PE cycle/row, half the DMA/SBUF of fp32),
accumulation in fp32 PSUM. Q^T/K^T/V stay SBUF-resident (no DRAM spill).

Device algorithm (per core):
  Phase A (per 512-token block, ko-major streamed): Q^T/K^T (head-dim on
           partitions) and V (tokens on partitions) accumulated over the 16
           contraction chunks; each arriving X chunk immediately feeds 8
           matmuls, so the PE is dense from ~12us (after the fixed NEFF
           prologue). Each V accumulation chain gets its own PSUM bank
           (start=True clears has_written for the whole bank). PSUM->SBUF
           copies are spread over ACT/DVE.
  Phase B (per (b, qb) window): three matmul chains interleaved at lags
           0/1/2 — score tiles S^T = K^T_chunk.T @ Q^T (keys on
           partitions), denominator ones-matmuls over E, and ctx^T @ V
           accumulation — 6 matmuls per step, which keeps the PE ahead of
           the ACT exp latency with only 2 score PSUM banks. E =
           exp(S^T/sqrt(hd)) on ACT (columns below the causal diagonal are
           skipped), causal 0/1 mask multiplies on diagonal tiles (DVE,
           also zeroing the skipped columns). Normalize via DVE
           reciprocal_approx_fast + multiply. No max-subtraction is
           needed: scores are O(5) so exp cannot overflow, and softmax is
           shift-invariant.
  Phase C: the out-projection for window qb is emitted inside window qb+1's
           chain steps (software pipelining hides both the softmax serial
           chain and the PSUM->SBUF cast latency of its results); casts are
           split between DVE and ACT according to each window's exp load.
           Partial outputs are written as bf16, alternating between both HW
           DMA queues (SP and ACT).
"""

import os

import numpy as np

import concourse.bacc as bacc
import concourse.tile as tile
from concourse import mybir
from concourse.bass_utils import run_bass_kernel_spmd


def _install_neff_cache():
    """Cache compiled NEFFs on disk keyed by BIR content hash.

    Purely a compile-time memo: identical BIR -> identical NEFF, so repeat
    runs skip the multi-minute neuronxcc compile. No effect on execution.
    """
    import hashlib
    import shutil

    import concourse.bass2jax as _b2j
    import concourse.bass_utils as _bu

    if getattr(_bu, "_neff_cache_installed", False):
        return
    cache_dir = os.environ.get("NEFF_CACHE_DIR", "/tmp/neff_cache")
    orig = _bu.compile_bir_kernel

    def cached(bir_json, tmpdir, neff_name="file.neff"):
        try:
            os.makedirs(cache_dir, exist_ok=True)
            key = hashlib.sha256(bir_json).hexdigest()[:24]
            cpath = os.path.join(cache_dir, key + ".neff")
            dst = os.path.join(tmpdir, neff_name)
            if os.path.exists(cpath):
                shutil.copy(cpath, dst)
                return dst
            out = orig(bir_json, tmpdir, neff_name)
            shutil.copy(out, cpath)
            return out
        except OSError:
            return orig(bir_json, tmpdir, neff_name)

    _bu.compile_bir_kernel = cached
    _b2j.compile_bir_kernel = cached
    _bu._neff_cache_installed = True


_install_neff_cache()

B, S, D, H = 2, 2048, 2048, 16
HD = D // H          # 128
NCORES = 8
HPC = H // NCORES    # heads per core = 2
T = B * S            # 4096 total token rows
KO = D // 128        # 16 contraction chunks
NTB = T // 512       # 8 phase-A token blocks of 512
NQB = S // 512       # 4 phase-B query blocks per batch
SCALE = 1.0 / float(np.sqrt(HD))

_built = {}


def _build(with_bias):
    f32 = mybir.dt.float32
    bf16 = mybir.dt.bfloat16

    nc = bacc.Bacc(None, target_bir_lowering=False)

    # ---- per-core DRAM parameters (host supplies per-core shards) ----
    xt_p = nc.declare_dram_parameter("XT", [KO, 128, T], bf16, False)
    wqt_p = nc.declare_dram_parameter("WQT", [KO, 128, HPC * HD], bf16, False)
    wkt_p = nc.declare_dram_parameter("WKT", [KO, 128, HPC * HD], bf16, False)
    wvt_p = nc.declare_dram_parameter("WVT", [KO, 128, HPC * HD], bf16, False)
    wot_p = nc.declare_dram_parameter("WOT", [128, HPC, D], bf16, False)
    bias_p = nc.declare_dram_parameter("BIAS", [1, 4, HPC * HD], bf16, False)
    mask_p = nc.declare_dram_parameter("MASK", [128, 4, 512], bf16, False)
    ones_p = nc.declare_dram_parameter("ONES", [128, 128], bf16, False)
    out_p = nc.declare_dram_parameter("OUT", [B, S, D], bf16, True)

    with tile.TileContext(nc) as tc:
        with tc.tile_pool(name="persist", bufs=1) as persist:
            qt_res = persist.tile([128, B, HPC, S], bf16)
            kt_res = persist.tile([128, B, HPC, S], bf16)
            # V natural layout: v_res[p, b, sc, h*HD+d] = V[b, 128*sc+p, h, d]
            v_res = persist.tile([128, B, S // 128, HPC * HD], bf16)
            wot = persist.tile([128, HPC, D], bf16)
            masks = persist.tile([128, 4, 512], bf16)
            onesr = persist.tile([128, 128], bf16)
            if with_bias:
                biasb = persist.tile([1, 4, HPC * HD], bf16)

            # ---------------- Phase A: projections ----------------
            with (
                tc.tile_pool(name="wqkv", bufs=1) as wpool,
                tc.tile_pool(name="xs", bufs=2) as xpool,
                tc.tile_pool(name="psQK", bufs=1, space="PSUM") as psQK,
                tc.tile_pool(name="psV", bufs=1, space="PSUM") as psV,
            ):
                wq = wpool.tile([128, KO, HPC * HD], bf16, tag="wq")
                wk = wpool.tile([128, KO, HPC * HD], bf16, tag="wk")
                wv = wpool.tile([128, KO, HPC * HD], bf16, tag="wv")
                # weight/constant loads ride the ACT hw queue; X rides
                # both queues. Interleave wq/wk/wv chunks so the ko-major
                # matmul stream (needs all three per ko) starts earliest.
                for g in range(4):
                    ksl = slice(g * 4, (g + 1) * 4)
                    for wt, wp in ((wq, wqt_p), (wk, wkt_p), (wv, wvt_p)):
                        nc.scalar.dma_start(
                            wt[:, ksl], wp[ksl].rearrange("ko p m -> p ko m")
                        )
                if with_bias:
                    nc.scalar.dma_start(biasb, bias_p[:])
                nc.scalar.dma_start(masks, mask_p[:])
                nc.scalar.dma_start(onesr, ones_p[:])
                nc.scalar.dma_start(wot, wot_p[:])
                warm = persist.tile([1, 8], f32)
                nc.vector.memset(warm, 1.0)
                nc.scalar.activation(
                    warm, warm, mybir.ActivationFunctionType.Exp
                )

                for tb in range(NTB):
                    b = (tb * 512) // S
                    s0 = (tb * 512) % S
                    sc0 = s0 // 128
                    xt = xpool.tile([128, KO, 512], bf16, tag="xt")
                    # tb0 streams 1-ko chunks (matches the 6-matmul/ko
                    # consumption rate at queue-ramp bandwidth); later tbs
                    # prefetch 4-ko chunks. Chunks alternate across the two
                    # HW DMA queues.
                    nch = 16 if tb == 0 else 4
                    kn = KO // nch
                    for g in range(nch):
                        ksl = slice(g * kn, (g + 1) * kn)
                        # the scalar queue carries ~3MB of weights first, so
                        # X rides sync-only until they drain (tb0/tb1)
                        eng = nc.sync if (tb < 2 or g % 2 == 0) else nc.scalar
                        eng.dma_start(
                            xt[:, ksl],
                            xt_p[ksl, :, tb * 512 : (tb + 1) * 512].rearrange(
                                "ko p t -> p ko t"
                            ),
                        )

                    psq = [
                        psQK.tile([128, 512], f32, tag=f"q{h}", name=f"psq{h}") for h in range(HPC)
                    ]
                    psk = [
                        psQK.tile([128, 512], f32, tag=f"k{h}", name=f"psk{h}") for h in range(HPC)
                    ]
                    # one PSUM bank per V accumulation chain: start=True
                    # clears has_written for the WHOLE bank, so chains must
                    # not share banks
                    psvs = [
                        psV.tile([128, 512], f32, tag=f"v{i}", name=f"psv{i}")
                        for i in range(4)
                    ]
                    # ko-major so each arriving X chunk feeds ~1.3us of PE
                    # work immediately (keeps the PE dense from t~2us).
                    for ko in range(KO):
                        st = ko == 0
                        sp = (ko == KO - 1) and not with_bias
                        for h in range(HPC):
                            nc.tensor.matmul(
                                psq[h],
                                lhsT=wq[:, ko, h * HD : (h + 1) * HD],
                                rhs=xt[:, ko],
                                start=st,
                                stop=sp,
                            )
                        for h in range(HPC):
                            nc.tensor.matmul(
                                psk[h],
                                lhsT=wk[:, ko, h * HD : (h + 1) * HD],
                                rhs=xt[:, ko],
                                start=st,
                                stop=sp,
                            )
                        for tsub in range(4):
                            nc.tensor.matmul(
                                psvs[tsub][:, :256],
                                lhsT=xt[:, ko, tsub * 128 : (tsub + 1) * 128],
                                rhs=wv[:, ko],
                                start=st,
                                stop=sp,
                            )
                    if with_bias:
                        ones512 = masks[0:1, 0, :]  # [1,512] of exact ones
                        for h in range(HPC):
                            nc.tensor.matmul(
                                psq[h],
                                lhsT=biasb[:, 0, h * HD : (h + 1) * HD],
                                rhs=ones512,
                                start=False,
                                stop=True,
                            )
                            nc.tensor.matmul(
                                psk[h],
                                lhsT=biasb[:, 1, h * HD : (h + 1) * HD],
                                rhs=ones512,
                                start=False,
                                stop=True,
                            )
                        for tsub in range(4):
                            nc.tensor.matmul(
                                psvs[tsub][:, :256],
                                lhsT=ones512[:, :128],
                                rhs=biasb[:, 2],
                                start=False,
                                stop=True,
                            )
                    # PSUM->SBUF copies spread across ACT + DVE (Pool
                    # cannot access PSUM)
                    for h in range(HPC):
                        nc.scalar.copy(qt_res[:, b, h, s0 : s0 + 512], psq[h])
                        nc.vector.tensor_copy(kt_res[:, b, h, s0 : s0 + 512], psk[h])
                    for tsub in range(4):
                        nc.vector.tensor_copy(
                            v_res[:, b, sc0 + tsub, :], psvs[tsub][:, :256]
                        )

            # ------------- Phase B + C: attention + out projection -------------
            with (
                tc.tile_pool(name="epool", bufs=34) as epool,
                tc.tile_pool(name="ctxp", bufs=2) as ctxp,
                tc.tile_pool(name="recp", bufs=3) as recp,
                tc.tile_pool(name="obp", bufs=4) as obp,
                tc.tile_pool(name="psS", bufs=2, space="PSUM") as psS,
                tc.tile_pool(name="psC", bufs=2, space="PSUM") as psC,
                tc.tile_pool(name="psD", bufs=2, space="PSUM") as psD,
                tc.tile_pool(name="psO", bufs=2, space="PSUM") as psO,
            ):
                dma_eng = [nc.sync, nc.scalar]

                def emit_outproj_group(b, qb, ctxs, g, cur_nk):
                    qc = 4 * qb + g // 4
                    oc = g % 4
                    pso = psO.tile([128, 512], f32, tag="o", name="pso")
                    for h in range(HPC):
                        nc.tensor.matmul(
                            pso,
                            lhsT=ctxs[h][:, qc * 128 : (qc + 1) * 128],
                            rhs=wot[:, h, oc * 512 : (oc + 1) * 512],
                            start=(h == 0),
                            stop=(h == HPC - 1),
                        )
                    ob = obp.tile([128, 512], bf16, tag="ob", name="ob")
                    # split the PSUM->SBUF casts between DVE and ACT by the
                    # surrounding window's load: in big windows ACT is
                    # exp-bound; in the exp-free tail (cur_nk=0) split evenly
                    if cur_nk == 0:
                        use_act = g % 2 == 1
                    elif cur_nk <= 8:
                        use_act = g % 3 == 2
                    else:
                        use_act = g % 4 == 3
                    if use_act:
                        nc.scalar.copy(ob, pso)
                    else:
                        nc.vector.tensor_copy(ob, pso)
                    dma_eng[(qc + oc) % 2].dma_start(
                        out_p[
                            b,
                            qc * 128 : (qc + 1) * 128,
                            oc * 512 : (oc + 1) * 512,
                        ],
                        ob,
                    )

                pending = None
                for b in range(B):
                    ctxs = [
                        ctxp.tile([128, S], bf16, tag=f"ctx{h}", name=f"ctx{h}") for h in range(HPC)
                    ]
                    for qb in range(NQB):
                        nk = 4 * (qb + 1)

                        def cut(t):
                            # diagonal tiles: queries < 128*i are fully
                            # masked for key chunk i -> skip those columns
                            return 128 * (t - 4 * qb) if t >= 4 * qb else 0

                        ess = [[], []]
                        pscs = [
                            psC.tile([128, 512], f32, tag="c", name=f"psc{h}") for h in range(HPC)
                        ]
                        psds = [
                            psD.tile([128, 512], f32, tag="d", name=f"psd{h}") for h in range(HPC)
                        ]
                        # Three interleaved chains at lags 0/1/2: score
                        # matmuls, denominator (ones) matmuls, ctx matmuls.
                        # 6 matmuls per step keep the PE busy past the exp
                        # latency, and the denominator rides the PE without
                        # a separate reduction pass.
                        steps = nk + 2
                        for t in range(steps):
                            # out-projection for the previous window rides
                            # between chain steps so its PSUM->SBUF cast
                            # latency hides behind chain matmuls
                            if pending is not None:
                                g0 = 16 * t // steps
                                g1 = 16 * (t + 1) // steps
                                for g in range(g0, g1):
                                    emit_outproj_group(*pending, g, nk)
                            if t < nk:
                                c0 = cut(t)
                                for h in range(HPC):
                                    pss = psS.tile([128, 512], f32, tag="s")
                                    nc.tensor.matmul(
                                        pss[:, c0:],
                                        lhsT=kt_res[:, b, h, t * 128 : (t + 1) * 128],
                                        rhs=qt_res[
                                            :, b, h,
                                            qb * 512 + c0 : (qb + 1) * 512,
                                        ],
                                        start=True,
                                        stop=True,
                                    )
                                    e = epool.tile([128, 512], bf16, tag="e")
                                    nc.scalar.activation(
                                        e[:, c0:], pss[:, c0:],
                                        mybir.ActivationFunctionType.Exp,
                                        scale=SCALE,
                                    )
                                    if t >= 4 * qb:
                                        # full-width: also zeroes the stale
                                        # (skipped) columns of the ring tile
                                        nc.vector.tensor_mul(
                                            e, e, masks[:, t - 4 * qb]
                                        )
                                    ess[h].append(e)
                            td = t - 1
                            if 0 <= td < nk:
                                c0 = cut(td)
                                for h in range(HPC):
                                    nc.tensor.matmul(
                                        psds[h][:, c0:],
                                        lhsT=onesr,
                                        rhs=ess[h][td][:, c0:],
                                        start=(td == 0),
                                        stop=(td == nk - 1),
                                    )
                            tc_ = t - 2
                            if 0 <= tc_ < nk:
                                c0 = cut(tc_)
                                for h in range(HPC):
                                    nc.tensor.matmul(
                                        pscs[h][:, c0:],
                                        lhsT=v_res[:, b, tc_, h * HD : (h + 1) * HD],
                                        rhs=ess[h][tc_][:, c0:],
                                        start=(tc_ == 0),
                                        stop=(tc_ == nk - 1),
                                    )
                        is_last = b == B - 1 and qb == NQB - 1
                        if is_last:
                            last_ps = (pscs, psds)
                        else:
                            # --- normalize ---
                            for h in range(HPC):
                                rec = recp.tile([128, 512], f32, tag="rec")
                                nc.vector.reciprocal_approx_fast(rec, psds[h])
                                nc.vector.tensor_mul(
                                    ctxs[h][:, qb * 512 : (qb + 1) * 512],
                                    pscs[h], rec,
                                )
                        pending = (b, qb, ctxs)
                # final window: normalize in 128-column chunks so its own
                # out-projection can start while later chunks still divide
                pscs, psds = last_ps
                b, qb, ctxs = pending
                recs = []
                for h in range(HPC):
                    rec = recp.tile([128, 512], f32, tag="rec")
                    nc.vector.reciprocal_approx_fast(rec, psds[h])
                    recs.append(rec)
                for cc in range(4):
                    csl = slice(cc * 128, (cc + 1) * 128)
                    osl = slice(qb * 512 + cc * 128, qb * 512 + (cc + 1) * 128)
                    for h in range(HPC):
                        nc.vector.tensor_mul(
                            ctxs[h][:, osl], pscs[h][:, csl], recs[h][:, csl]
                        )
                    for g in range(4 * cc, 4 * cc + 4):
                        emit_outproj_group(b, qb, ctxs, g, 0)

    nc.finalize()
    return nc


def _get_nc(with_bias=False):
    if with_bias not in _built:
        _built[with_bias] = _build(with_bias)
    return _built[with_bias]


def kernel(hidden_states, attention_mask, Wq, bq, Wk, bk, Wv, bv, Wo, bo):
    import ml_dtypes

    bf16 = ml_dtypes.bfloat16

    hidden_states = np.asarray(hidden_states, dtype=np.float32)
    Wq, Wk, Wv, Wo = (np.asarray(w, dtype=np.float32) for w in (Wq, Wk, Wv, Wo))
    bq, bk, bv, bo = (np.asarray(v, dtype=np.float32) for v in (bq, bk, bv, bo))

    with_bias = bool(np.any(bq) or np.any(bk) or np.any(bv))

    x = hidden_states.reshape(T, D)
    # [KO, 128, T]: XT[ko, p, t] = x[t, 128*ko + p]
    xt = np.ascontiguousarray(x.T).reshape(KO, 128, T).astype(bf16)

    # causal 0/1 masks for the 4 diagonal-tile offsets: mask[p, i, f] = p + 128*i <= f
    p_idx = np.arange(128)[:, None, None]
    i_idx = np.arange(4)[None, :, None]
    f_idx = np.arange(512)[None, None, :]
    mask = (p_idx + 128 * i_idx <= f_idx).astype(bf16)
    ones = np.ones((128, 128), dtype=bf16)

    in_maps = []
    for c in range(NCORES):
        rows = slice(c * HPC * HD, (c + 1) * HPC * HD)
        wqt = np.ascontiguousarray(Wq[rows, :].T).reshape(KO, 128, HPC * HD)
        wkt = np.ascontiguousarray(Wk[rows, :].T).reshape(KO, 128, HPC * HD)
        wvt = np.ascontiguousarray(Wv[rows, :].T).reshape(KO, 128, HPC * HD)
        # WOT[p, h, n] = Wo[n, c*256 + h*128 + p]
        wot = np.ascontiguousarray(
            Wo[:, rows].T.reshape(HPC, 128, D).transpose(1, 0, 2)
        )
        bias = np.stack([bq[rows], bk[rows], bv[rows], np.zeros(HPC * HD, np.float32)])[
            None
        ]
        in_maps.append(
            {
                "XT": xt,
                "WQT": wqt.astype(bf16),
                "WKT": wkt.astype(bf16),
                "WVT": wvt.astype(bf16),
                "WOT": wot.astype(bf16),
                "BIAS": bias.astype(bf16),
                "MASK": mask,
                "ONES": ones,
            }
        )

    res = run_bass_kernel_spmd(_get_nc(with_bias), in_maps, list(range(NCORES)))
    out = res.results[0]["OUT"].astype(np.float32)
    for c in range(1, NCORES):
        out += res.results[c]["OUT"].astype(np.float32)
    out += bo
    return out
